# revision 14
# baseline (speedup 1.0000x reference)
"""Trainium2 Bass kernel for a dense transformer decoder block (B=4, T=2048,
C=1024, 16 heads x 64, DFF=4096), SPMD across 8 NeuronCores.

Sharding: core pair (2b, 2b+1) shares batch element b. Token blocks (128) are
interleaved between the pair so both cores see near-equal causal windows:
core par owns global blocks {2t+par}. The host permutes each core's token
order so OWN blocks always sit at odd positions 2t+1 -> one uniform SPMD
program. Causal masks for the last two window blocks of each query block are
per-core DATA (m1: zeros for par=0 / ones for par=1; m2: triu for both).

Pipeline: LN1+V -> per-pair K/Q || attention(chunk0) -> Wo/LN2(c0) ->
FFN(tch0) || attention(chunk1) -> Wo/LN2(c1) -> FFN(tch1). The softmax exp
(scalar engine) hides behind FFN/projection matmuls on the tensor engine.

All matmuls bf16 (fp32 PSUM); LN stats, softmax normalization, residuals fp32.
LN gamma/beta folded into adjacent weights on host.
"""

import os
from contextlib import ExitStack

os.environ.setdefault("MYCRO_LOCAL_CACHE", "1")

import numpy as np
import ml_dtypes

import concourse.bacc as bacc
import concourse.bass as bass
import concourse.mybir as mybir
import concourse.tile as tile
from concourse.bass_utils import run_bass_kernel_spmd

BF16 = ml_dtypes.bfloat16
P = 128
C = 1024
NPAIR = 8   # head pairs
NKT = 8     # C / 128 contraction tiles
NW = 16     # window token blocks (2048 tokens)
NT = 8      # own token blocks (1024 tokens)
ND = 32     # DFF / 128 tiles
EPS = 1e-5

f32 = mybir.dt.float32
bf16 = mybir.dt.bfloat16
FT = mybir.ActivationFunctionType
ALU = mybir.AluOpType


def _build(flags):
    nc = bacc.Bacc("TRN2", target_bir_lowering=False, debug=False, num_devices=8)

    xkv = nc.dram_tensor("xkv", [2048, C], f32, kind="ExternalInput")
    wq = nc.dram_tensor("wq", [P, 8192], bf16, kind="ExternalInput")
    wk = nc.dram_tensor("wk", [P, 8192], bf16, kind="ExternalInput")
    wv = nc.dram_tensor("wv", [P, 8192], bf16, kind="ExternalInput")
    wo = nc.dram_tensor("wo", [P, 8192], bf16, kind="ExternalInput")
    w1 = nc.dram_tensor("w1", [P, 32768], bf16, kind="ExternalInput")
    w2 = nc.dram_tensor("w2", [P, 32768], bf16, kind="ExternalInput")
    masks = nc.dram_tensor("masks", [P, 2 * P], bf16, kind="ExternalInput")
    identd = nc.dram_tensor("identd", [P, P], bf16, kind="ExternalInput")
    qbias = nc.dram_tensor("qbias", [P, NPAIR], f32, kind="ExternalInput")
    kbias = nc.dram_tensor("kbias", [P, NPAIR], f32, kind="ExternalInput")
    b1p = nc.dram_tensor("b1p", [P, ND], f32, kind="ExternalInput")
    bo_row = nc.dram_tensor("bo_row", [P, C], f32, kind="ExternalInput")
    b2_row = nc.dram_tensor("b2_row", [P, C], f32, kind="ExternalInput")
    out = nc.dram_tensor("out", [1024, C], bf16, kind="ExternalOutput")

    with tile.TileContext(nc) as tc, ExitStack() as es:
        consts = es.enter_context(tc.tile_pool(name="consts", bufs=1))
        mk_sb = consts.tile([P, 2 * P], bf16, tag="mk")
        nc.sync.dma_start(out=mk_sb[:, :], in_=masks.ap()[:, :])
        ident_sb = consts.tile([P, P], bf16, tag="ident")
        nc.sync.dma_start(out=ident_sb[:, :], in_=identd.ap()[:, :])
        qb_sb = consts.tile([P, NPAIR], f32, tag="qb")
        nc.sync.dma_start(out=qb_sb[:, :], in_=qbias.ap()[:, :])
        kb_sb = consts.tile([P, NPAIR], f32, tag="kb")
        nc.sync.dma_start(out=kb_sb[:, :], in_=kbias.ap()[:, :])
        b1_sb = consts.tile([P, ND], f32, tag="b1")
        nc.sync.dma_start(out=b1_sb[:, :], in_=b1p.ap()[:, :])
        eps_sb = consts.tile([P, 1], f32, tag="eps")
        nc.vector.memset(eps_sb[:, :], EPS)
        if flags["bo"]:
            bo_sb = consts.tile([P, C], f32, tag="bo")
            nc.sync.dma_start(out=bo_sb[:, :], in_=bo_row.ap()[:, :])
        if flags["b2"]:
            b2_sb = consts.tile([P, C], f32, tag="b2")
            nc.sync.dma_start(out=b2_sb[:, :], in_=b2_row.ap()[:, :])

        # persistent activation storage
        qt_pool = es.enter_context(tc.tile_pool(name="qt", bufs=NPAIR))
        kt_pool = es.enter_context(tc.tile_pool(name="kt", bufs=NPAIR))
        v_pool = es.enter_context(tc.tile_pool(name="vv", bufs=NW))
        x_pool = es.enter_context(tc.tile_pool(name="xx", bufs=NT))
        o_pool = es.enter_context(tc.tile_pool(name="oo", bufs=4, side="right"))
        QT = [qt_pool.tile([P, 1024], bf16, tag="qt", name=f"qt{i}") for i in range(NPAIR)]
        KT = [kt_pool.tile([P, 2048], bf16, tag="kt", name=f"kt{i}") for i in range(NPAIR)]
        # V with interleaved ones columns: per pair 65+65 cols
        VO = [v_pool.tile([P, NPAIR * 130], bf16, tag="vv", name=f"vo{i}") for i in range(NW)]
        X = [x_pool.tile([P, C], bf16, tag="xx", name=f"xt{i}") for i in range(NT)]
        O = [o_pool.tile([P, C], bf16, tag="oo", name=f"ot{i}") for i in range(NT)]

        def ln_tile(src_ap, lnp, zpool):
            """LayerNorm a [128, C] fp32 tile -> bf16 z tile (g/b folded out)."""
            if isinstance(src_ap, tuple):
                xw = lnp.tile([P, C], f32, tag="xw")
                nc.sync.dma_start(out=xw[:, :], in_=src_ap[0])
            else:
                xw = src_ap
            stats = lnp.tile([P, 2, 6], f32, tag="stats")
            nc.vector.bn_stats(out=stats[:, 0, :], in_=xw[:, 0:512])
            nc.vector.bn_stats(out=stats[:, 1, :], in_=xw[:, 512:1024])
            mv = lnp.tile([P, 2], f32, tag="mv")
            nc.vector.bn_aggr(out=mv[:, :], in_=stats[:, :, :])
            rsig = lnp.tile([P, 1], f32, tag="rsig")
            nc.scalar.activation(rsig[:, :], mv[:, 1:2], FT.Sqrt,
                                 bias=eps_sb[:, :], scale=1.0)
            nc.vector.reciprocal(rsig[:, :], rsig[:, :])
            z = zpool.tile([P, C], bf16, tag="z")
            nc.vector.tensor_scalar(z[:, :], xw[:, :], mv[:, 0:1], rsig[:, :],
                                    ALU.subtract, ALU.mult)
            return z

        # ---------------- Phase A: LN1, hT, V projection ----------------
        ht_es = ExitStack()
        htp = ht_es.enter_context(tc.tile_pool(name="ht", bufs=1))
        HT = [htp.tile([P, 2048], bf16, tag=f"ht{i}", name=f"ht{i}") for i in range(NKT)]
        with tc.tile_pool(name="ln1", bufs=3) as lnp, \
             tc.tile_pool(name="z1", bufs=3) as zpool, \
             tc.tile_pool(name="tps1", bufs=2, space="PSUM") as tps1, \
             tc.tile_pool(name="wvp", bufs=NKT) as wv_pool, \
             tc.tile_pool(name="wka", bufs=3) as wka_pool, \
             tc.tile_pool(name="kqa", bufs=2, space="PSUM") as kqa, \
             tc.tile_pool(name="qkvps", bufs=2, space="PSUM") as qkvps:
            WV = [wv_pool.tile([P, 1024], bf16, tag="wv", name=f"wvt{i}") for i in range(NKT)]
            for kt in range(NKT):
                nc.sync.dma_start(out=WV[kt][:, :],
                                  in_=wv.ap()[:, kt * 1024:(kt + 1) * 1024])

            def k_step(wh, pr):
                wk_sb = wka_pool.tile([P, 1024], bf16, tag="wka", name="wka")
                nc.sync.dma_start(out=wk_sb[:, :],
                                  in_=wk.ap()[:, pr * 1024:(pr + 1) * 1024])
                pk = kqa.tile([P, 512], f32, tag="kqa", name="pk")
                for kt in range(NKT):
                    nc.tensor.matmul(
                        pk[:, :], wk_sb[:, kt * P:(kt + 1) * P],
                        HT[kt][:, wh * 512:(wh + 1) * 512],
                        start=(kt == 0), stop=(kt == NKT - 1))
                nc.vector.tensor_scalar_add(
                    KT[pr][:, wh * 512:(wh + 1) * 512], pk[:, :],
                    kb_sb[:, pr:pr + 1])

            for w in range(NW):
                z = ln_tile((xkv.ap()[w * P:(w + 1) * P, :],), lnp, zpool)
                for c in range(NKT):
                    tp = tps1.tile([P, P], bf16, tag="tp")
                    nc.tensor.transpose(tp[:, :], z[:, c * P:(c + 1) * P],
                                        ident_sb[:, :])
                    nc.scalar.copy(
                        out=HT[c][:, w * P:(w + 1) * P], in_=tp[:, :])
                pv = qkvps.tile([P, 1024], f32, tag="qkvps")
                for kt in range(NKT):
                    for hf in range(2):
                        nc.tensor.matmul(
                            pv[:, hf * 512:(hf + 1) * 512],
                            HT[kt][:, w * P:(w + 1) * P],
                            WV[kt][:, hf * 512:(hf + 1) * 512],
                            start=(kt == 0), stop=(kt == NKT - 1))
                vdst = VO[w][:, :].rearrange("p (pr hi dd) -> p pr hi dd",
                                             pr=NPAIR, hi=2)[:, :, :, 0:64]
                vsrc = pv[:, :].rearrange("p (pr hi dd) -> p pr hi dd",
                                          pr=NPAIR, hi=2)
                nc.scalar.copy(out=vdst, in_=vsrc)
                ones = VO[w][:, :].rearrange("p (pr hi dd) -> p pr hi dd",
                                             pr=NPAIR, hi=2)[:, :, :, 64:65]
                nc.vector.memset(ones, 1.0)
                if 3 <= w <= 14:
                    wh = (w - 3) // 4
                    for pr in (2 * ((w - 3) % 4), 2 * ((w - 3) % 4) + 1):
                        k_step(wh, pr)
                elif w == 15:
                    for pr in range(NPAIR):
                        k_step(3, pr)
            # prefetch own-token residual rows (own = odd permuted blocks)
            for t in range(NT):
                xf = lnp.tile([P, C], f32, tag="xw", name="xf")
                nc.sync.dma_start(
                    out=xf[:, :],
                    in_=xkv.ap()[(2 * t + 1) * P:(2 * t + 2) * P, :])
                nc.vector.tensor_copy(out=X[t][:, :], in_=xf[:, :])

        # ---------------- attention (chunk c, head pair pr) ----------------
        attn_es = ExitStack()
        sps = attn_es.enter_context(tc.tile_pool(name="sps", bufs=2, space="PSUM"))
        ops_pool = attn_es.enter_context(tc.tile_pool(name="ops", bufs=2, space="PSUM"))
        ep_pool = attn_es.enter_context(tc.tile_pool(name="epp", bufs=3, side="right"))
        sal_pool = attn_es.enter_context(tc.tile_pool(name="sal", bufs=4, side="right"))
        wo_pool = attn_es.enter_context(tc.tile_pool(name="wos", bufs=1, side="right"))
        wo_sb = wo_pool.tile([P, 8192], bf16, tag="wo")
        nc.sync.dma_start(out=wo_sb[:, :], in_=wo.ap()[:, :])

        def attn_pr_steps(c, pr):
            """List of closures: full attention of chunk c for head pair pr."""
            state = {}

            def alloc():
                state[0] = ops_pool.tile([P, 260], f32, tag="ops", name="opsA")
                state[1] = ops_pool.tile([P, 260], f32, tag="ops", name="opsB")

            def do_j(j):
                q0 = max(0, (j // 2 - 4 * c)) * P
                qlen = 512 - q0
                sp = sps.tile([P, 1024], f32, tag="sps")
                for hi in range(2):
                    nc.tensor.matmul(
                        sp[:, hi * 512:hi * 512 + qlen],
                        KT[pr][hi * 64:(hi + 1) * 64, j * P:(j + 1) * P],
                        QT[pr][hi * 64:(hi + 1) * 64, c * 512 + q0:(c + 1) * 512],
                        start=True, stop=True)
                ep = ep_pool.tile([P, 1024], bf16, tag="ep")
                spv = sp[:, :].rearrange("p (hi q) -> p hi q", hi=2)[:, :, 0:qlen]
                epv = ep[:, 0:2 * qlen].rearrange("p (hi q) -> p hi q", hi=2)
                nc.scalar.activation(epv, spv, FT.Exp)
                t_d = j // 2
                if t_d >= 4 * c:
                    off = (t_d - 4 * c) * P - q0
                    mcol = (j % 2) * P
                    for hi in range(2):
                        sl = ep[:, hi * qlen + off:hi * qlen + off + P]
                        nc.vector.tensor_mul(sl, sl, mk_sb[:, mcol:mcol + P])
                for t in range(max(4 * c, j // 2), 4 * c + 4):
                    tl = t - 4 * c
                    gt = state[tl // 2]
                    gc = (tl % 2) * 130
                    off = tl * P - q0
                    # one start / one stop per psum bank (accumulation group)
                    for hi in range(2):
                        nc.tensor.matmul(
                            gt[:, gc + hi * 65:gc + (hi + 1) * 65],
                            ep[:, hi * qlen + off:hi * qlen + off + P],
                            VO[j][:, pr * 130 + hi * 65:pr * 130 + (hi + 1) * 65],
                            start=(j == 0 and hi == 0 and tl % 2 == 0),
                            stop=(hi == 1 and
                                  ((j == 8 * c + 3 and tl == 1) or
                                   (j == 8 * c + 7 and tl == 3))))

            def norm():
                for tl in range(4):
                    t = 4 * c + tl
                    gt = state[tl // 2]
                    gc = (tl % 2) * 130
                    rs = sal_pool.tile([P, 2], f32, tag="rs")
                    rsrc = gt[:, gc:gc + 130].rearrange(
                        "p (hi d) -> p hi d", hi=2)[:, :, 64:65]
                    nc.vector.reciprocal(rs[:, :], rsrc)
                    for hi in range(2):
                        nc.vector.tensor_scalar_mul(
                            O[t][:, pr * P + hi * 64:pr * P + hi * 64 + 64],
                            gt[:, gc + hi * 65:gc + hi * 65 + 64],
                            rs[:, hi:hi + 1])

            steps = [alloc]
            steps += [(lambda j=j: do_j(j)) for j in range(8 * c + 8)]
            steps.append(norm)
            return steps

        # ---- Phase B: Q projection per pair + attention chunk 0 ----
        with tc.tile_pool(name="wqkv", bufs=2) as wqkv_pool, \
             tc.tile_pool(name="kqps", bufs=2, space="PSUM") as kqps:
            for pr in range(NPAIR):
                wq_sb = wqkv_pool.tile([P, 1024], bf16, tag="wqk")
                nc.sync.dma_start(out=wq_sb[:, :],
                                  in_=wq.ap()[:, pr * 1024:(pr + 1) * 1024])
                for qh in range(2):
                    pq = kqps.tile([P, 512], f32, tag="kq")
                    for kt in range(NKT):
                        qrhs = HT[kt][:, :].rearrange(
                            "p (t par d) -> p t par d", t=8, par=2)[
                                :, qh * 4:(qh + 1) * 4, 1:2, :]
                        nc.tensor.matmul(
                            pq[:, :], wq_sb[:, kt * P:(kt + 1) * P],
                            qrhs, start=(kt == 0), stop=(kt == NKT - 1))
                    nc.vector.tensor_scalar(
                        QT[pr][:, qh * 512:(qh + 1) * 512], pq[:, :],
                        qb_sb[:, pr:pr + 1], 0.125, ALU.add, ALU.mult)
                for s in attn_pr_steps(0, pr):
                    s()
        ht_es.close()

        # ---- shared FFN/Wo/transpose psum pool + weight streams ----
        fps = attn_es.enter_context(tc.tile_pool(name="fps", bufs=2, space="PSUM"))
        ot_pool = attn_es.enter_context(tc.tile_pool(name="otp", bufs=2))
        ln2p = attn_es.enter_context(tc.tile_pool(name="ln2", bufs=2))
        z2pool = attn_es.enter_context(tc.tile_pool(name="z2", bufs=2))
        h2t_pool = attn_es.enter_context(tc.tile_pool(name="h2t", bufs=1))
        ut_pool = attn_es.enter_context(tc.tile_pool(name="ut", bufs=ND))
        w1_pool = attn_es.enter_context(tc.tile_pool(name="w1s", bufs=3))
        w2_pool = attn_es.enter_context(tc.tile_pool(name="w2s", bufs=4))
        H2T = [h2t_pool.tile([P, 1024], bf16, tag=f"h2t{i}", name=f"h2t{i}")
               for i in range(NKT)]
        UT = [ut_pool.tile([P, 512], bf16, tag="ut", name=f"ut{i}") for i in range(ND)]

        def wo_t(t):
            """O[t] -> OT (xbar dma transpose) -> Wo -> X[t] residual."""
            ot = ot_pool.tile([P, 1024], bf16, tag="ot")
            for kt in range(NKT):
                qeng = nc.scalar if kt % 2 else nc.sync
                qeng.dma_start_transpose(
                    out=ot[:, kt * P:(kt + 1) * P],
                    in_=O[t][:, kt * P:(kt + 1) * P])
            for hf in range(2):
                pw = sps.tile([P, 512], f32, tag="sps", name="pw")
                for kt in range(NKT):
                    nc.tensor.matmul(
                        pw[:, :], ot[:, kt * P:(kt + 1) * P],
                        wo_sb[:, kt * 1024 + hf * 512:kt * 1024 + (hf + 1) * 512],
                        start=(kt == 0), stop=(kt == NKT - 1))
                xsl = X[t][:, hf * 512:(hf + 1) * 512]
                nc.vector.tensor_add(xsl, pw[:, :], xsl)
                if flags["bo"]:
                    nc.vector.tensor_add(xsl, xsl, bo_sb[:, hf * 512:(hf + 1) * 512])

        def ln2_t(t):
            """X[t] -> LN2 -> z2 -> H2T columns (xbar dma transpose)."""
            z2 = ln_tile(X[t], ln2p, z2pool)
            c, tl = t // 4, t % 4
            for kt in range(NKT):
                qeng = nc.scalar if kt % 2 else nc.sync
                qeng.dma_start_transpose(
                    out=H2T[kt][:, c * 512 + tl * P:c * 512 + (tl + 1) * P],
                    in_=z2[:, kt * P:(kt + 1) * P])

        def ffn_steps(tch, chs=(0, 1), do_w1=True, dual_q=False):
            """Closures for FFN on token chunk tch (512 tokens)."""
            steps = []
            state = {}

            def w1_d(d):
                w1_sb = w1_pool.tile([P, 1024], bf16, tag="w1")
                nc.sync.dma_start(out=w1_sb[:, :],
                                  in_=w1.ap()[:, d * 1024:(d + 1) * 1024])
                pu = fps.tile([P, 512], f32, tag="fps")
                for kt in range(NKT):
                    nc.tensor.matmul(
                        pu[:, :], w1_sb[:, kt * P:(kt + 1) * P],
                        H2T[kt][:, tch * 512:(tch + 1) * 512],
                        start=(kt == 0), stop=(kt == NKT - 1))
                nc.scalar.activation(UT[d][:, :], pu[:, :], FT.Relu,
                                     bias=b1_sb[:, d:d + 1], scale=1.0)

            def w2_start():
                state["y0"] = fps.tile([P, 512], f32, tag="fps", name="y0")
                state["y1"] = fps.tile([P, 512], f32, tag="fps", name="y1")

            def w2_dp(ch, tp_, dp):
                w2_sb = w2_pool.tile([P, 2, 512], bf16, tag="w2")
                qeng = nc.scalar if (dual_q and dp % 2) else nc.sync
                qeng.dma_start(
                    out=w2_sb[:, :, :],
                    in_=w2.ap()[:, :].rearrange("p (d n) -> p d n", d=ND)[
                        :, 2 * dp:2 * dp + 2, ch * 512:(ch + 1) * 512])
                for dk in range(2):
                    d = 2 * dp + dk
                    for k in range(2):
                        tt = tp_ * 2 + k
                        nc.tensor.matmul(
                            state[f"y{k}"][:, :],
                            UT[d][:, tt * P:(tt + 1) * P], w2_sb[:, dk:dk + 1, :],
                            start=(d == 0), stop=(d == ND - 1))

            def w2_end(ch, tp_):
                for k in range(2):
                    t = tch * 4 + tp_ * 2 + k
                    xsl = X[t][:, ch * 512:(ch + 1) * 512]
                    nc.vector.tensor_add(xsl, state[f"y{k}"][:, :], xsl)
                    if flags["b2"]:
                        nc.vector.tensor_add(
                            xsl, xsl, b2_sb[:, ch * 512:(ch + 1) * 512])

            if do_w1:
                for d in range(ND):
                    steps.append(lambda d=d: w1_d(d))
            for ch in chs:
                for tp_ in range(2):
                    steps.append(w2_start)
                    for dp in range(ND // 2):
                        steps.append(lambda ch=ch, tp_=tp_, dp=dp: w2_dp(ch, tp_, dp))
                    steps.append(lambda ch=ch, tp_=tp_: w2_end(ch, tp_))
            return steps

        # ---- Phase C: Wo + LN2 for chunk 0, software-pipelined ----
        wo_t(0)
        wo_t(1)
        ln2_t(0)
        wo_t(2)
        ln2_t(1)
        wo_t(3)
        ln2_t(2)
        ln2_t(3)

        # ---- Phase D: FFN(tch0, W1+W2ch0) interleaved with attn chunk 1 ----
        f_steps = ffn_steps(0, chs=(0,))
        a_steps = []
        for pr in range(NPAIR):
            a_steps += attn_pr_steps(1, pr)
        fi = ai = 0
        while fi < len(f_steps) or ai < len(a_steps):
            if fi < len(f_steps):
                f_steps[fi]()
                fi += 1
            if ai < len(a_steps):
                a_steps[ai]()
                ai += 1
            if ai < len(a_steps):
                a_steps[ai]()
                ai += 1

        # ---- Phase E: Wo + LN2 chunk 1 (sps psum) || FFN0 W2 ch1 (fps) ----
        e_steps = [lambda: wo_t(4), lambda: wo_t(5), lambda: ln2_t(4),
                   lambda: wo_t(6), lambda: ln2_t(5), lambda: wo_t(7),
                   lambda: ln2_t(6), lambda: ln2_t(7)]
        f2_steps = ffn_steps(0, chs=(1,), do_w1=False, dual_q=True)
        fi = ei = 0
        while fi < len(f2_steps) or ei < len(e_steps):
            for _ in range(5):
                if fi < len(f2_steps):
                    f2_steps[fi]()
                    fi += 1
            if ei < len(e_steps):
                e_steps[ei]()
                ei += 1
        for t in range(4):
            nc.sync.dma_start(out=out.ap()[t * P:(t + 1) * P, :], in_=X[t][:, :])

        # ---- Phase F: FFN(tch1): W1 pipelined with W2ch0, then W2ch1 ----
        tch = 1

        def f_w1(d):
            w1_sb = w1_pool.tile([P, 1024], bf16, tag="w1")
            nc.sync.dma_start(out=w1_sb[:, :],
                              in_=w1.ap()[:, d * 1024:(d + 1) * 1024])
            pu = fps.tile([P, 512], f32, tag="fps")
            for kt in range(NKT):
                nc.tensor.matmul(
                    pu[:, :], w1_sb[:, kt * P:(kt + 1) * P],
                    H2T[kt][:, tch * 512:(tch + 1) * 512],
                    start=(kt == 0), stop=(kt == NKT - 1))
            nc.scalar.activation(UT[d][:, :], pu[:, :], FT.Relu,
                                 bias=b1_sb[:, d:d + 1], scale=1.0)

        def f_w2grp(ch, d, ya, yb, w2_sb, dk, dq=False):
            for k in range(2):
                for tp_ in range(2):
                    tt = tp_ * 2 + k
                    y = (ya, yb)[tp_]
                    nc.tensor.matmul(
                        y[:, k * 512:(k + 1) * 512],
                        UT[d][:, tt * P:(tt + 1) * P], w2_sb[:, dk:dk + 1, :],
                        start=(d == 0), stop=(d == ND - 1))

        def f_w2dma(ch, dp, dq=False):
            w2_sb = w2_pool.tile([P, 2, 512], bf16, tag="w2")
            qeng = nc.scalar if (dq and dp % 2) else nc.sync
            qeng.dma_start(
                out=w2_sb[:, :, :],
                in_=w2.ap()[:, :].rearrange("p (d n) -> p d n", d=ND)[
                    :, 2 * dp:2 * dp + 2, ch * 512:(ch + 1) * 512])
            return w2_sb

        def f_w2end(ch, ya, yb):
            for tp_ in range(2):
                for k in range(2):
                    t = tch * 4 + tp_ * 2 + k
                    xsl = X[t][:, ch * 512:(ch + 1) * 512]
                    nc.vector.tensor_add(xsl, (ya, yb)[tp_][:, k * 512:(k + 1) * 512], xsl)
                    if flags["b2"]:
                        nc.vector.tensor_add(
                            xsl, xsl, b2_sb[:, ch * 512:(ch + 1) * 512])

        ya = sps.tile([P, 1024], f32, tag="sps", name="ya")
        yb = sps.tile([P, 1024], f32, tag="sps", name="yb")
        f_w1(0)
        f_w1(1)
        w2t = {}
        for d in range(ND):
            if d % 2 == 0:
                w2t[d // 2] = f_w2dma(0, d // 2)
            f_w2grp(0, d, ya, yb, w2t[d // 2], d % 2)
            if d + 2 < ND:
                f_w1(d + 2)
        f_w2end(0, ya, yb)
        ya2 = sps.tile([P, 1024], f32, tag="sps", name="ya2")
        yb2 = sps.tile([P, 1024], f32, tag="sps", name="yb2")
        for dp in range(ND // 2):
            w2_sb = f_w2dma(1, dp, dq=True)
            for dk in range(2):
                f_w2grp(1, 2 * dp + dk, ya2, yb2, w2_sb, dk)
        f_w2end(1, ya2, yb2)
        for t in range(4, 8):
            nc.sync.dma_start(out=out.ap()[t * P:(t + 1) * P, :], in_=X[t][:, :])
        attn_es.close()

    nc.compile()
    return nc


_CACHE = {}


def _prep(inputs):
    """Host-side preprocessing: fold LN affine into weights, tile/cast, shard."""
    x = np.asarray(inputs["x"], np.float32)
    Wq = np.asarray(inputs["Wq"], np.float32)
    Wk = np.asarray(inputs["Wk"], np.float32)
    Wv = np.asarray(inputs["Wv"], np.float32)
    Wo = np.asarray(inputs["Wo"], np.float32)
    bo = np.asarray(inputs["bo"], np.float32)
    W1 = np.asarray(inputs["W1"], np.float32)
    b1 = np.asarray(inputs["b1"], np.float32)
    W2 = np.asarray(inputs["W2"], np.float32)
    b2 = np.asarray(inputs["b2"], np.float32)
    g1 = np.asarray(inputs["g1"], np.float32)
    be1 = np.asarray(inputs["be1"], np.float32)
    g2 = np.asarray(inputs["g2"], np.float32)
    be2 = np.asarray(inputs["be2"], np.float32)

    Wq_g = (Wq * g1[None, :, None]).astype(BF16)   # [16,1024,64]
    Wk_g = (Wk * g1[None, :, None]).astype(BF16)
    Wv_g = (Wv * g1[None, :, None]).astype(BF16)
    qb = np.einsum('c,hcd->hd', be1, Wq_g.astype(np.float32))  # [16,64]
    kb = np.einsum('c,hcd->hd', be1, Wk_g.astype(np.float32))
    vb = np.einsum('c,hcd->hd', be1, Wv_g.astype(np.float32))
    if np.abs(vb).max() > 0:
        raise NotImplementedError("nonzero folded V bias not supported")

    def lhsT_pack(wflat):  # [1024 c, 1024 m] -> [128, (pair, kt, 128)]
        return np.ascontiguousarray(
            wflat.reshape(8, 128, 8, 128).transpose(1, 2, 0, 3).reshape(128, 8192))

    def rhs_pack(wflat):   # [1024 k, 1024 n] -> [128, (kt, 1024)]
        return np.ascontiguousarray(
            wflat.reshape(8, 128, 1024).transpose(1, 0, 2).reshape(128, 8192))

    wq_h = lhsT_pack(Wq_g.transpose(1, 0, 2).reshape(1024, 1024))
    wk_h = lhsT_pack(Wk_g.transpose(1, 0, 2).reshape(1024, 1024))
    wv_h = rhs_pack(Wv_g.transpose(1, 0, 2).reshape(1024, 1024))
    wo_h = rhs_pack(Wo.astype(BF16))
    W1_g = (W1 * g2[:, None]).astype(BF16)         # [1024, 4096]
    b1p = b1 + be2 @ W1_g.astype(np.float32)
    w1_h = np.ascontiguousarray(
        W1_g.reshape(8, 128, 32, 128).transpose(1, 2, 0, 3).reshape(128, 32768))
    w2_h = np.ascontiguousarray(
        W2.astype(BF16).reshape(32, 128, 1024).transpose(1, 0, 2).reshape(128, 32768))

    qb_t = np.zeros((128, 8), np.float32)
    kb_t = np.zeros((128, 8), np.float32)
    for pr in range(8):
        qb_t[0:64, pr] = qb[2 * pr]
        qb_t[64:128, pr] = qb[2 * pr + 1]
        kb_t[0:64, pr] = kb[2 * pr]
        kb_t[64:128, pr] = kb[2 * pr + 1]
    b1_t = np.ascontiguousarray(b1p.reshape(32, 128).T.astype(np.float32))
    bo_t = np.broadcast_to(bo, (128, 1024)).astype(np.float32).copy()
    b2_t = np.broadcast_to(b2, (128, 1024)).astype(np.float32).copy()

    triu = np.triu(np.ones((128, 128), np.float32))
    ident = np.eye(128, dtype=np.float32).astype(BF16)

    flags = {"bo": bool(np.abs(bo).max() > 0), "b2": bool(np.abs(b2).max() > 0)}

    shared = dict(wq=wq_h, wk=wk_h, wv=wv_h, wo=wo_h, w1=w1_h, w2=w2_h,
                  identd=ident, qbias=qb_t, kbias=kb_t,
                  b1p=b1_t, bo_row=bo_t, b2_row=b2_t)
    in_maps = []
    for core in range(8):
        b, par = core // 2, core % 2
        xb = x[b].reshape(16, 128, 1024)
        if par == 0:
            # swap even/odd blocks so own (even-global) blocks sit at odd slots
            perm = [i + 1 if i % 2 == 0 else i - 1 for i in range(16)]
            xw = np.ascontiguousarray(xb[perm].reshape(2048, 1024))
            m1 = np.zeros((128, 128), np.float32)
        else:
            xw = np.ascontiguousarray(xb.reshape(2048, 1024))
            m1 = np.ones((128, 128), np.float32)
        mk = np.concatenate([m1, triu], axis=1).astype(BF16)
        in_maps.append({"xkv": xw, "masks": mk, **shared})
    return in_maps, flags


def _get_nc(flags):
    key = tuple(sorted(flags.items()))
    if key not in _CACHE:
        _CACHE[key] = _build(flags)
    return _CACHE[key]


def run(inputs, **kw):
    in_maps, flags = _prep(inputs)
    nc = _get_nc(flags)
    res = run_bass_kernel_spmd(nc, in_maps, core_ids=list(range(8)), **kw)
    x = np.asarray(inputs["x"], np.float32)
    outf = np.zeros_like(x)
    for core in range(8):
        b, par = core // 2, core % 2
        r = np.asarray(res.results[core]["out"], np.float32)
        for t in range(8):
            g = 2 * t + par
            outf[b, g * 128:(g + 1) * 128] = r[t * 128:(t + 1) * 128]
    return outf, res


def kernel(**inputs):
    outf, _ = run(inputs)
    return outf


# revision 15
# speedup vs baseline: 1.1937x; 1.1937x over previous
"""Trainium2 Bass kernel for a dense transformer decoder block (B=4, T=2048,
C=1024, 16 heads x 64, DFF=4096), SPMD across 8 NeuronCores.

Sharding: core pair (2b, 2b+1) shares batch element b. Token blocks (128) are
interleaved between the pair so both cores see near-equal causal windows:
core par owns global blocks {2t+par}. The host permutes each core's token
order so OWN blocks always sit at odd positions 2t+1 -> one uniform SPMD
program. Causal masks for the last two window blocks of each query block are
per-core DATA (m1: zeros for par=0 / ones for par=1; m2: triu for both).

Pipeline: LN1+V -> per-pair K/Q || attention(chunk0) -> Wo/LN2(c0) ->
FFN(tch0) || attention(chunk1) -> Wo/LN2(c1) -> FFN(tch1). The softmax exp
(scalar engine) hides behind FFN/projection matmuls on the tensor engine.

All matmuls bf16 (fp32 PSUM); LN stats, softmax normalization, residuals fp32.
LN gamma/beta folded into adjacent weights on host.
"""

import os
from contextlib import ExitStack

os.environ.setdefault("MYCRO_LOCAL_CACHE", "1")

import numpy as np
import ml_dtypes

import concourse.bacc as bacc
import concourse.bass as bass
import concourse.mybir as mybir
import concourse.tile as tile
from concourse.bass_utils import run_bass_kernel_spmd

BF16 = ml_dtypes.bfloat16
P = 128
C = 1024
NPAIR = 8   # head pairs
NKT = 8     # C / 128 contraction tiles
NW = 16     # window token blocks (2048 tokens)
NT = 8      # own token blocks (1024 tokens)
ND = 32     # DFF / 128 tiles
EPS = 1e-5

f32 = mybir.dt.float32
bf16 = mybir.dt.bfloat16
FT = mybir.ActivationFunctionType
ALU = mybir.AluOpType


def _build(flags):
    nc = bacc.Bacc("TRN2", target_bir_lowering=False, debug=False, num_devices=8)

    xkv = nc.dram_tensor("xkv", [2048, C], f32, kind="ExternalInput")
    wq = nc.dram_tensor("wq", [P, 8192], bf16, kind="ExternalInput")
    wk = nc.dram_tensor("wk", [P, 8192], bf16, kind="ExternalInput")
    wv = nc.dram_tensor("wv", [P, 8192], bf16, kind="ExternalInput")
    wo = nc.dram_tensor("wo", [P, 8192], bf16, kind="ExternalInput")
    w1 = nc.dram_tensor("w1", [P, 32768], bf16, kind="ExternalInput")
    w2 = nc.dram_tensor("w2", [P, 32768], bf16, kind="ExternalInput")
    masks = nc.dram_tensor("masks", [P, 2 * P], bf16, kind="ExternalInput")
    identd = nc.dram_tensor("identd", [P, P], bf16, kind="ExternalInput")
    qbias = nc.dram_tensor("qbias", [P, NPAIR], f32, kind="ExternalInput")
    kbias = nc.dram_tensor("kbias", [P, NPAIR], f32, kind="ExternalInput")
    b1p = nc.dram_tensor("b1p", [P, ND], f32, kind="ExternalInput")
    bo_row = nc.dram_tensor("bo_row", [P, C], f32, kind="ExternalInput")
    b2_row = nc.dram_tensor("b2_row", [P, C], f32, kind="ExternalInput")
    out = nc.dram_tensor("out", [1024, C], bf16, kind="ExternalOutput")

    with tile.TileContext(nc) as tc, ExitStack() as es:
        consts = es.enter_context(tc.tile_pool(name="consts", bufs=1))
        mk_sb = consts.tile([P, 2 * P], bf16, tag="mk")
        nc.sync.dma_start(out=mk_sb[:, :], in_=masks.ap()[:, :])
        ident_sb = consts.tile([P, P], bf16, tag="ident")
        nc.sync.dma_start(out=ident_sb[:, :], in_=identd.ap()[:, :])
        qb_sb = consts.tile([P, NPAIR], f32, tag="qb")
        nc.sync.dma_start(out=qb_sb[:, :], in_=qbias.ap()[:, :])
        kb_sb = consts.tile([P, NPAIR], f32, tag="kb")
        nc.sync.dma_start(out=kb_sb[:, :], in_=kbias.ap()[:, :])
        b1_sb = consts.tile([P, ND], f32, tag="b1")
        nc.sync.dma_start(out=b1_sb[:, :], in_=b1p.ap()[:, :])
        eps_sb = consts.tile([P, 1], f32, tag="eps")
        nc.vector.memset(eps_sb[:, :], EPS)
        if flags["bo"]:
            bo_sb = consts.tile([P, C], f32, tag="bo")
            nc.sync.dma_start(out=bo_sb[:, :], in_=bo_row.ap()[:, :])
        if flags["b2"]:
            b2_sb = consts.tile([P, C], f32, tag="b2")
            nc.sync.dma_start(out=b2_sb[:, :], in_=b2_row.ap()[:, :])

        # persistent activation storage
        qt_pool = es.enter_context(tc.tile_pool(name="qt", bufs=NPAIR))
        kt_pool = es.enter_context(tc.tile_pool(name="kt", bufs=NPAIR))
        v_pool = es.enter_context(tc.tile_pool(name="vv", bufs=NW))
        x_pool = es.enter_context(tc.tile_pool(name="xx", bufs=NT))
        o_pool = es.enter_context(tc.tile_pool(name="oo", bufs=4, side="right"))
        QT = [qt_pool.tile([P, 1024], bf16, tag="qt", name=f"qt{i}") for i in range(NPAIR)]
        KT = [kt_pool.tile([P, 2048], bf16, tag="kt", name=f"kt{i}") for i in range(NPAIR)]
        # V with interleaved ones columns: per pair 65+65 cols
        VO = [v_pool.tile([P, NPAIR * 130], bf16, tag="vv", name=f"vo{i}") for i in range(NW)]
        X = [x_pool.tile([P, C], bf16, tag="xx", name=f"xt{i}") for i in range(NT)]
        O = [o_pool.tile([P, C], bf16, tag="oo", name=f"ot{i}") for i in range(NT)]

        def ln_tile(src_ap, lnp, zpool):
            """LayerNorm a [128, C] fp32 tile -> bf16 z tile (g/b folded out)."""
            if isinstance(src_ap, tuple):
                xw = lnp.tile([P, C], f32, tag="xw")
                nc.sync.dma_start(out=xw[:, :], in_=src_ap[0])
            else:
                xw = src_ap
            stats = lnp.tile([P, 2, 6], f32, tag="stats")
            nc.vector.bn_stats(out=stats[:, 0, :], in_=xw[:, 0:512])
            nc.vector.bn_stats(out=stats[:, 1, :], in_=xw[:, 512:1024])
            mv = lnp.tile([P, 2], f32, tag="mv")
            nc.vector.bn_aggr(out=mv[:, :], in_=stats[:, :, :])
            rsig = lnp.tile([P, 1], f32, tag="rsig")
            nc.scalar.activation(rsig[:, :], mv[:, 1:2], FT.Sqrt,
                                 bias=eps_sb[:, :], scale=1.0)
            nc.vector.reciprocal(rsig[:, :], rsig[:, :])
            z = zpool.tile([P, C], bf16, tag="z")
            nc.vector.tensor_scalar(z[:, :], xw[:, :], mv[:, 0:1], rsig[:, :],
                                    ALU.subtract, ALU.mult)
            return z

        # ---------------- Phase A: LN1, hT, V projection ----------------
        ht_es = ExitStack()
        htp = ht_es.enter_context(tc.tile_pool(name="ht", bufs=1))
        HT = [htp.tile([P, 2048], bf16, tag=f"ht{i}", name=f"ht{i}") for i in range(NKT)]
        with tc.tile_pool(name="ln1", bufs=3) as lnp, \
             tc.tile_pool(name="z1", bufs=3) as zpool, \
             tc.tile_pool(name="tps1", bufs=2, space="PSUM") as tps1, \
             tc.tile_pool(name="wvp", bufs=NKT) as wv_pool, \
             tc.tile_pool(name="wka", bufs=3) as wka_pool, \
             tc.tile_pool(name="kqa", bufs=2, space="PSUM") as kqa, \
             tc.tile_pool(name="qkvps", bufs=2, space="PSUM") as qkvps:
            WV = [wv_pool.tile([P, 1024], bf16, tag="wv", name=f"wvt{i}") for i in range(NKT)]
            for kt in range(NKT):
                nc.sync.dma_start(out=WV[kt][:, :],
                                  in_=wv.ap()[:, kt * 1024:(kt + 1) * 1024])

            def k_step(wh, pr):
                wk_sb = wka_pool.tile([P, 1024], bf16, tag="wka", name="wka")
                nc.sync.dma_start(out=wk_sb[:, :],
                                  in_=wk.ap()[:, pr * 1024:(pr + 1) * 1024])
                pk = kqa.tile([P, 512], f32, tag="kqa", name="pk")
                for kt in range(NKT):
                    nc.tensor.matmul(
                        pk[:, :], wk_sb[:, kt * P:(kt + 1) * P],
                        HT[kt][:, wh * 512:(wh + 1) * 512],
                        start=(kt == 0), stop=(kt == NKT - 1))
                nc.vector.tensor_scalar_add(
                    KT[pr][:, wh * 512:(wh + 1) * 512], pk[:, :],
                    kb_sb[:, pr:pr + 1])

            for w in range(NW):
                z = ln_tile((xkv.ap()[w * P:(w + 1) * P, :],), lnp, zpool)
                for c in range(NKT):
                    tp = tps1.tile([P, P], bf16, tag="tp")
                    nc.tensor.transpose(tp[:, :], z[:, c * P:(c + 1) * P],
                                        ident_sb[:, :])
                    nc.scalar.copy(
                        out=HT[c][:, w * P:(w + 1) * P], in_=tp[:, :])
                pv = qkvps.tile([P, 1024], f32, tag="qkvps")
                for kt in range(NKT):
                    for hf in range(2):
                        nc.tensor.matmul(
                            pv[:, hf * 512:(hf + 1) * 512],
                            HT[kt][:, w * P:(w + 1) * P],
                            WV[kt][:, hf * 512:(hf + 1) * 512],
                            start=(kt == 0), stop=(kt == NKT - 1))
                vdst = VO[w][:, :].rearrange("p (pr hi dd) -> p pr hi dd",
                                             pr=NPAIR, hi=2)[:, :, :, 0:64]
                vsrc = pv[:, :].rearrange("p (pr hi dd) -> p pr hi dd",
                                          pr=NPAIR, hi=2)
                nc.scalar.copy(out=vdst, in_=vsrc)
                ones = VO[w][:, :].rearrange("p (pr hi dd) -> p pr hi dd",
                                             pr=NPAIR, hi=2)[:, :, :, 64:65]
                nc.vector.memset(ones, 1.0)
                if 3 <= w <= 14:
                    wh = (w - 3) // 4
                    for pr in (2 * ((w - 3) % 4), 2 * ((w - 3) % 4) + 1):
                        k_step(wh, pr)
                elif w == 15:
                    for pr in range(NPAIR):
                        k_step(3, pr)
            # prefetch own-token residual rows (own = odd permuted blocks)
            for t in range(NT):
                xf = lnp.tile([P, C], f32, tag="xw", name="xf")
                nc.sync.dma_start(
                    out=xf[:, :],
                    in_=xkv.ap()[(2 * t + 1) * P:(2 * t + 2) * P, :])
                nc.vector.tensor_copy(out=X[t][:, :], in_=xf[:, :])

        # ---------------- attention (chunk c, head pair pr) ----------------
        attn_es = ExitStack()
        sps = attn_es.enter_context(tc.tile_pool(name="sps", bufs=2, space="PSUM"))
        ops_pool = attn_es.enter_context(tc.tile_pool(name="ops", bufs=2, space="PSUM"))
        ep_pool = attn_es.enter_context(tc.tile_pool(name="epp", bufs=3, side="right"))
        sal_pool = attn_es.enter_context(tc.tile_pool(name="sal", bufs=4, side="right"))
        wo_pool = attn_es.enter_context(tc.tile_pool(name="wos", bufs=1, side="right"))
        wo_sb = wo_pool.tile([P, 8192], bf16, tag="wo")
        nc.sync.dma_start(out=wo_sb[:, :], in_=wo.ap()[:, :])

        def attn_pr_steps(c, pr):
            """List of closures: full attention of chunk c for head pair pr."""
            state = {}

            def alloc():
                state[0] = ops_pool.tile([P, 260], f32, tag="ops", name="opsA")
                state[1] = ops_pool.tile([P, 260], f32, tag="ops", name="opsB")

            def do_j(j):
                q0 = max(0, (j // 2 - 4 * c)) * P
                qlen = 512 - q0
                sp = sps.tile([P, 1024], f32, tag="sps")
                for hi in range(2):
                    nc.tensor.matmul(
                        sp[:, hi * 512:hi * 512 + qlen],
                        KT[pr][hi * 64:(hi + 1) * 64, j * P:(j + 1) * P],
                        QT[pr][hi * 64:(hi + 1) * 64, c * 512 + q0:(c + 1) * 512],
                        start=True, stop=True)
                ep = ep_pool.tile([P, 1024], bf16, tag="ep")
                spv = sp[:, :].rearrange("p (hi q) -> p hi q", hi=2)[:, :, 0:qlen]
                epv = ep[:, 0:2 * qlen].rearrange("p (hi q) -> p hi q", hi=2)
                nc.scalar.activation(epv, spv, FT.Exp)
                t_d = j // 2
                if t_d >= 4 * c:
                    off = (t_d - 4 * c) * P - q0
                    mcol = (j % 2) * P
                    for hi in range(2):
                        sl = ep[:, hi * qlen + off:hi * qlen + off + P]
                        nc.vector.tensor_mul(sl, sl, mk_sb[:, mcol:mcol + P])
                for t in range(max(4 * c, j // 2), 4 * c + 4):
                    tl = t - 4 * c
                    gt = state[tl // 2]
                    gc = (tl % 2) * 130
                    off = tl * P - q0
                    # one start / one stop per psum bank (accumulation group)
                    for hi in range(2):
                        nc.tensor.matmul(
                            gt[:, gc + hi * 65:gc + (hi + 1) * 65],
                            ep[:, hi * qlen + off:hi * qlen + off + P],
                            VO[j][:, pr * 130 + hi * 65:pr * 130 + (hi + 1) * 65],
                            start=(j == 0 and hi == 0 and tl % 2 == 0),
                            stop=(hi == 1 and
                                  ((j == 8 * c + 3 and tl == 1) or
                                   (j == 8 * c + 7 and tl == 3))))

            def norm():
                for tl in range(4):
                    t = 4 * c + tl
                    gt = state[tl // 2]
                    gc = (tl % 2) * 130
                    rs = sal_pool.tile([P, 2], f32, tag="rs")
                    rsrc = gt[:, gc:gc + 130].rearrange(
                        "p (hi d) -> p hi d", hi=2)[:, :, 64:65]
                    nc.vector.reciprocal(rs[:, :], rsrc)
                    for hi in range(2):
                        nc.vector.tensor_scalar_mul(
                            O[t][:, pr * P + hi * 64:pr * P + hi * 64 + 64],
                            gt[:, gc + hi * 65:gc + hi * 65 + 64],
                            rs[:, hi:hi + 1])

            steps = [alloc]
            steps += [(lambda j=j: do_j(j)) for j in range(8 * c + 8)]
            steps.append(norm)
            return steps

        # ---- Phase B: Q projection per pair + attention chunk 0 ----
        with tc.tile_pool(name="wqkv", bufs=2) as wqkv_pool, \
             tc.tile_pool(name="kqps", bufs=2, space="PSUM") as kqps:
            for pr in range(NPAIR):
                wq_sb = wqkv_pool.tile([P, 1024], bf16, tag="wqk")
                nc.sync.dma_start(out=wq_sb[:, :],
                                  in_=wq.ap()[:, pr * 1024:(pr + 1) * 1024])
                for qh in range(2):
                    pq = kqps.tile([P, 512], f32, tag="kq")
                    for kt in range(NKT):
                        qrhs = HT[kt][:, :].rearrange(
                            "p (t par d) -> p t par d", t=8, par=2)[
                                :, qh * 4:(qh + 1) * 4, 1:2, :]
                        nc.tensor.matmul(
                            pq[:, :], wq_sb[:, kt * P:(kt + 1) * P],
                            qrhs, start=(kt == 0), stop=(kt == NKT - 1))
                    nc.vector.tensor_scalar(
                        QT[pr][:, qh * 512:(qh + 1) * 512], pq[:, :],
                        qb_sb[:, pr:pr + 1], 0.125, ALU.add, ALU.mult)
                for s in attn_pr_steps(0, pr):
                    s()
        ht_es.close()

        # ---- shared FFN/Wo/transpose psum pool + weight streams ----
        fps = attn_es.enter_context(tc.tile_pool(name="fps", bufs=2, space="PSUM"))
        ot_pool = attn_es.enter_context(tc.tile_pool(name="otp", bufs=2))
        ln2p = attn_es.enter_context(tc.tile_pool(name="ln2", bufs=2))
        z2pool = attn_es.enter_context(tc.tile_pool(name="z2", bufs=2))
        h2t_pool = attn_es.enter_context(tc.tile_pool(name="h2t", bufs=1))
        ut_pool = attn_es.enter_context(tc.tile_pool(name="ut", bufs=ND))
        w1_pool = attn_es.enter_context(tc.tile_pool(name="w1s", bufs=3))
        w2_pool = attn_es.enter_context(tc.tile_pool(name="w2s", bufs=4))
        H2T = [h2t_pool.tile([P, 1024], bf16, tag=f"h2t{i}", name=f"h2t{i}")
               for i in range(NKT)]
        UT = [ut_pool.tile([P, 512], bf16, tag="ut", name=f"ut{i}") for i in range(ND)]

        def wo_t(t):
            """O[t] -> OT -> Wo -> X[t] residual (sps-pool psum)."""
            ot = ot_pool.tile([P, 1024], bf16, tag="ot")
            for kt in range(NKT):
                tp = sps.tile([P, 512], bf16, tag="sps", name="tpo")
                nc.tensor.transpose(tp[:, 0:P], O[t][:, kt * P:(kt + 1) * P],
                                    ident_sb[:, :])
                nc.scalar.copy(out=ot[:, kt * P:(kt + 1) * P],
                               in_=tp[:, 0:P])
            for hf in range(2):
                pw = sps.tile([P, 512], f32, tag="sps", name="pw")
                for kt in range(NKT):
                    nc.tensor.matmul(
                        pw[:, :], ot[:, kt * P:(kt + 1) * P],
                        wo_sb[:, kt * 1024 + hf * 512:kt * 1024 + (hf + 1) * 512],
                        start=(kt == 0), stop=(kt == NKT - 1))
                xsl = X[t][:, hf * 512:(hf + 1) * 512]
                nc.vector.tensor_add(xsl, pw[:, :], xsl)
                if flags["bo"]:
                    nc.vector.tensor_add(xsl, xsl, bo_sb[:, hf * 512:(hf + 1) * 512])

        def ln2_t(t):
            """X[t] -> LN2 -> z2 -> H2T columns (sps-pool psum)."""
            z2 = ln_tile(X[t], ln2p, z2pool)
            c, tl = t // 4, t % 4
            for kt in range(NKT):
                tp = sps.tile([P, 512], bf16, tag="sps", name="tpz")
                nc.tensor.transpose(tp[:, 0:P], z2[:, kt * P:(kt + 1) * P],
                                    ident_sb[:, :])
                nc.scalar.copy(
                    out=H2T[kt][:, c * 512 + tl * P:c * 512 + (tl + 1) * P],
                    in_=tp[:, 0:P])

        def ffn_steps(tch, chs=(0, 1), do_w1=True, dual_q=False):
            """Closures for FFN on token chunk tch (512 tokens)."""
            steps = []
            state = {}

            def w1_d(d):
                w1_sb = w1_pool.tile([P, 1024], bf16, tag="w1")
                nc.sync.dma_start(out=w1_sb[:, :],
                                  in_=w1.ap()[:, d * 1024:(d + 1) * 1024])
                pu = fps.tile([P, 512], f32, tag="fps")
                for kt in range(NKT):
                    nc.tensor.matmul(
                        pu[:, :], w1_sb[:, kt * P:(kt + 1) * P],
                        H2T[kt][:, tch * 512:(tch + 1) * 512],
                        start=(kt == 0), stop=(kt == NKT - 1))
                nc.scalar.activation(UT[d][:, :], pu[:, :], FT.Relu,
                                     bias=b1_sb[:, d:d + 1], scale=1.0)

            def w2_start():
                state["y0"] = fps.tile([P, 512], f32, tag="fps", name="y0")
                state["y1"] = fps.tile([P, 512], f32, tag="fps", name="y1")

            def w2_dp(ch, tp_, dp):
                w2_sb = w2_pool.tile([P, 2, 512], bf16, tag="w2")
                qeng = nc.scalar if (dual_q and dp % 2) else nc.sync
                qeng.dma_start(
                    out=w2_sb[:, :, :],
                    in_=w2.ap()[:, :].rearrange("p (d n) -> p d n", d=ND)[
                        :, 2 * dp:2 * dp + 2, ch * 512:(ch + 1) * 512])
                for dk in range(2):
                    d = 2 * dp + dk
                    for k in range(2):
                        tt = tp_ * 2 + k
                        nc.tensor.matmul(
                            state[f"y{k}"][:, :],
                            UT[d][:, tt * P:(tt + 1) * P], w2_sb[:, dk:dk + 1, :],
                            start=(d == 0), stop=(d == ND - 1))

            def w2_end(ch, tp_):
                for k in range(2):
                    t = tch * 4 + tp_ * 2 + k
                    xsl = X[t][:, ch * 512:(ch + 1) * 512]
                    nc.vector.tensor_add(xsl, state[f"y{k}"][:, :], xsl)
                    if flags["b2"]:
                        nc.vector.tensor_add(
                            xsl, xsl, b2_sb[:, ch * 512:(ch + 1) * 512])

            if do_w1:
                for d in range(ND):
                    steps.append(lambda d=d: w1_d(d))
            for ch in chs:
                for tp_ in range(2):
                    steps.append(w2_start)
                    for dp in range(ND // 2):
                        steps.append(lambda ch=ch, tp_=tp_, dp=dp: w2_dp(ch, tp_, dp))
                    steps.append(lambda ch=ch, tp_=tp_: w2_end(ch, tp_))
            return steps

        # ---- Phase C: Wo + LN2 for chunk 0, software-pipelined ----
        wo_t(0)
        wo_t(1)
        ln2_t(0)
        wo_t(2)
        ln2_t(1)
        wo_t(3)
        ln2_t(2)
        ln2_t(3)

        # ---- Phase D: FFN(tch0, W1+W2ch0) interleaved with attn chunk 1 ----
        f_steps = ffn_steps(0, chs=(0,))
        a_steps = []
        for pr in range(NPAIR):
            a_steps += attn_pr_steps(1, pr)
        fi = ai = 0
        while fi < len(f_steps) or ai < len(a_steps):
            if fi < len(f_steps):
                f_steps[fi]()
                fi += 1
            if ai < len(a_steps):
                a_steps[ai]()
                ai += 1
            if ai < len(a_steps):
                a_steps[ai]()
                ai += 1

        # ---- Phase E: Wo + LN2 chunk 1 (sps psum) || FFN0 W2 ch1 (fps) ----
        e_steps = [lambda: wo_t(4), lambda: wo_t(5), lambda: ln2_t(4),
                   lambda: wo_t(6), lambda: ln2_t(5), lambda: wo_t(7),
                   lambda: ln2_t(6), lambda: ln2_t(7)]
        f2_steps = ffn_steps(0, chs=(1,), do_w1=False, dual_q=True)
        fi = ei = 0
        while fi < len(f2_steps) or ei < len(e_steps):
            for _ in range(5):
                if fi < len(f2_steps):
                    f2_steps[fi]()
                    fi += 1
            if ei < len(e_steps):
                e_steps[ei]()
                ei += 1
        for t in range(4):
            nc.sync.dma_start(out=out.ap()[t * P:(t + 1) * P, :], in_=X[t][:, :])

        # ---- Phase F: FFN(tch1): W1 pipelined with W2ch0, then W2ch1 ----
        tch = 1

        def f_w1(d):
            w1_sb = w1_pool.tile([P, 1024], bf16, tag="w1")
            nc.sync.dma_start(out=w1_sb[:, :],
                              in_=w1.ap()[:, d * 1024:(d + 1) * 1024])
            pu = fps.tile([P, 512], f32, tag="fps")
            for kt in range(NKT):
                nc.tensor.matmul(
                    pu[:, :], w1_sb[:, kt * P:(kt + 1) * P],
                    H2T[kt][:, tch * 512:(tch + 1) * 512],
                    start=(kt == 0), stop=(kt == NKT - 1))
            nc.scalar.activation(UT[d][:, :], pu[:, :], FT.Relu,
                                 bias=b1_sb[:, d:d + 1], scale=1.0)

        def f_w2grp(ch, d, ya, yb, w2_sb, dk, dq=False):
            for k in range(2):
                for tp_ in range(2):
                    tt = tp_ * 2 + k
                    y = (ya, yb)[tp_]
                    nc.tensor.matmul(
                        y[:, k * 512:(k + 1) * 512],
                        UT[d][:, tt * P:(tt + 1) * P], w2_sb[:, dk:dk + 1, :],
                        start=(d == 0), stop=(d == ND - 1))

        def f_w2dma(ch, dp, dq=False):
            w2_sb = w2_pool.tile([P, 2, 512], bf16, tag="w2")
            qeng = nc.scalar if (dq and dp % 2) else nc.sync
            qeng.dma_start(
                out=w2_sb[:, :, :],
                in_=w2.ap()[:, :].rearrange("p (d n) -> p d n", d=ND)[
                    :, 2 * dp:2 * dp + 2, ch * 512:(ch + 1) * 512])
            return w2_sb

        def f_w2end(ch, ya, yb):
            for tp_ in range(2):
                for k in range(2):
                    t = tch * 4 + tp_ * 2 + k
                    xsl = X[t][:, ch * 512:(ch + 1) * 512]
                    nc.vector.tensor_add(xsl, (ya, yb)[tp_][:, k * 512:(k + 1) * 512], xsl)
                    if flags["b2"]:
                        nc.vector.tensor_add(
                            xsl, xsl, b2_sb[:, ch * 512:(ch + 1) * 512])

        ya = sps.tile([P, 1024], f32, tag="sps", name="ya")
        yb = sps.tile([P, 1024], f32, tag="sps", name="yb")
        f_w1(0)
        f_w1(1)
        w2t = {}
        for d in range(ND):
            if d % 2 == 0:
                w2t[d // 2] = f_w2dma(0, d // 2)
            f_w2grp(0, d, ya, yb, w2t[d // 2], d % 2)
            if d + 2 < ND:
                f_w1(d + 2)
        f_w2end(0, ya, yb)
        ya2 = sps.tile([P, 1024], f32, tag="sps", name="ya2")
        yb2 = sps.tile([P, 1024], f32, tag="sps", name="yb2")
        for dp in range(ND // 2):
            w2_sb = f_w2dma(1, dp, dq=True)
            for dk in range(2):
                f_w2grp(1, 2 * dp + dk, ya2, yb2, w2_sb, dk)
        f_w2end(1, ya2, yb2)
        for t in range(4, 8):
            nc.sync.dma_start(out=out.ap()[t * P:(t + 1) * P, :], in_=X[t][:, :])
        attn_es.close()

    nc.compile()
    return nc


_CACHE = {}


def _prep(inputs):
    """Host-side preprocessing: fold LN affine into weights, tile/cast, shard."""
    x = np.asarray(inputs["x"], np.float32)
    Wq = np.asarray(inputs["Wq"], np.float32)
    Wk = np.asarray(inputs["Wk"], np.float32)
    Wv = np.asarray(inputs["Wv"], np.float32)
    Wo = np.asarray(inputs["Wo"], np.float32)
    bo = np.asarray(inputs["bo"], np.float32)
    W1 = np.asarray(inputs["W1"], np.float32)
    b1 = np.asarray(inputs["b1"], np.float32)
    W2 = np.asarray(inputs["W2"], np.float32)
    b2 = np.asarray(inputs["b2"], np.float32)
    g1 = np.asarray(inputs["g1"], np.float32)
    be1 = np.asarray(inputs["be1"], np.float32)
    g2 = np.asarray(inputs["g2"], np.float32)
    be2 = np.asarray(inputs["be2"], np.float32)

    Wq_g = (Wq * g1[None, :, None]).astype(BF16)   # [16,1024,64]
    Wk_g = (Wk * g1[None, :, None]).astype(BF16)
    Wv_g = (Wv * g1[None, :, None]).astype(BF16)
    qb = np.einsum('c,hcd->hd', be1, Wq_g.astype(np.float32))  # [16,64]
    kb = np.einsum('c,hcd->hd', be1, Wk_g.astype(np.float32))
    vb = np.einsum('c,hcd->hd', be1, Wv_g.astype(np.float32))
    if np.abs(vb).max() > 0:
        raise NotImplementedError("nonzero folded V bias not supported")

    def lhsT_pack(wflat):  # [1024 c, 1024 m] -> [128, (pair, kt, 128)]
        return np.ascontiguousarray(
            wflat.reshape(8, 128, 8, 128).transpose(1, 2, 0, 3).reshape(128, 8192))

    def rhs_pack(wflat):   # [1024 k, 1024 n] -> [128, (kt, 1024)]
        return np.ascontiguousarray(
            wflat.reshape(8, 128, 1024).transpose(1, 0, 2).reshape(128, 8192))

    wq_h = lhsT_pack(Wq_g.transpose(1, 0, 2).reshape(1024, 1024))
    wk_h = lhsT_pack(Wk_g.transpose(1, 0, 2).reshape(1024, 1024))
    wv_h = rhs_pack(Wv_g.transpose(1, 0, 2).reshape(1024, 1024))
    wo_h = rhs_pack(Wo.astype(BF16))
    W1_g = (W1 * g2[:, None]).astype(BF16)         # [1024, 4096]
    b1p = b1 + be2 @ W1_g.astype(np.float32)
    w1_h = np.ascontiguousarray(
        W1_g.reshape(8, 128, 32, 128).transpose(1, 2, 0, 3).reshape(128, 32768))
    w2_h = np.ascontiguousarray(
        W2.astype(BF16).reshape(32, 128, 1024).transpose(1, 0, 2).reshape(128, 32768))

    qb_t = np.zeros((128, 8), np.float32)
    kb_t = np.zeros((128, 8), np.float32)
    for pr in range(8):
        qb_t[0:64, pr] = qb[2 * pr]
        qb_t[64:128, pr] = qb[2 * pr + 1]
        kb_t[0:64, pr] = kb[2 * pr]
        kb_t[64:128, pr] = kb[2 * pr + 1]
    b1_t = np.ascontiguousarray(b1p.reshape(32, 128).T.astype(np.float32))
    bo_t = np.broadcast_to(bo, (128, 1024)).astype(np.float32).copy()
    b2_t = np.broadcast_to(b2, (128, 1024)).astype(np.float32).copy()

    triu = np.triu(np.ones((128, 128), np.float32))
    ident = np.eye(128, dtype=np.float32).astype(BF16)

    flags = {"bo": bool(np.abs(bo).max() > 0), "b2": bool(np.abs(b2).max() > 0)}

    shared = dict(wq=wq_h, wk=wk_h, wv=wv_h, wo=wo_h, w1=w1_h, w2=w2_h,
                  identd=ident, qbias=qb_t, kbias=kb_t,
                  b1p=b1_t, bo_row=bo_t, b2_row=b2_t)
    in_maps = []
    for core in range(8):
        b, par = core // 2, core % 2
        xb = x[b].reshape(16, 128, 1024)
        if par == 0:
            # swap even/odd blocks so own (even-global) blocks sit at odd slots
            perm = [i + 1 if i % 2 == 0 else i - 1 for i in range(16)]
            xw = np.ascontiguousarray(xb[perm].reshape(2048, 1024))
            m1 = np.zeros((128, 128), np.float32)
        else:
            xw = np.ascontiguousarray(xb.reshape(2048, 1024))
            m1 = np.ones((128, 128), np.float32)
        mk = np.concatenate([m1, triu], axis=1).astype(BF16)
        in_maps.append({"xkv": xw, "masks": mk, **shared})
    return in_maps, flags


def _get_nc(flags):
    key = tuple(sorted(flags.items()))
    if key not in _CACHE:
        _CACHE[key] = _build(flags)
    return _CACHE[key]


def run(inputs, **kw):
    in_maps, flags = _prep(inputs)
    nc = _get_nc(flags)
    res = run_bass_kernel_spmd(nc, in_maps, core_ids=list(range(8)), **kw)
    x = np.asarray(inputs["x"], np.float32)
    outf = np.zeros_like(x)
    for core in range(8):
        b, par = core // 2, core % 2
        r = np.asarray(res.results[core]["out"], np.float32)
        for t in range(8):
            g = 2 * t + par
            outf[b, g * 128:(g + 1) * 128] = r[t * 128:(t + 1) * 128]
    return outf, res


def kernel(**inputs):
    outf, _ = run(inputs)
    return outf


# revision 16
# speedup vs baseline: 1.2091x; 1.0129x over previous
"""Trainium2 Bass kernel for a dense transformer decoder block (B=4, T=2048,
C=1024, 16 heads x 64, DFF=4096), SPMD across 8 NeuronCores.

Sharding: core pair (2b, 2b+1) shares batch element b. Token blocks (128) are
interleaved between the pair so both cores see near-equal causal windows:
core par owns global blocks {2t+par}. The host permutes each core's token
order so OWN blocks always sit at odd positions 2t+1 -> one uniform SPMD
program. Causal masks for the last two window blocks of each query block are
per-core DATA (m1: zeros for par=0 / ones for par=1; m2: triu for both).

Pipeline: LN1+V -> per-pair K/Q || attention(chunk0) -> Wo/LN2(c0) ->
FFN(tch0) || attention(chunk1) -> Wo/LN2(c1) -> FFN(tch1). The softmax exp
(scalar engine) hides behind FFN/projection matmuls on the tensor engine.

All matmuls bf16 (fp32 PSUM); LN stats, softmax normalization, residuals fp32.
LN gamma/beta folded into adjacent weights on host.
"""

import os
from contextlib import ExitStack

os.environ.setdefault("MYCRO_LOCAL_CACHE", "1")

import numpy as np
import ml_dtypes

import concourse.bacc as bacc
import concourse.bass as bass
import concourse.mybir as mybir
import concourse.tile as tile
from concourse.bass_utils import run_bass_kernel_spmd

BF16 = ml_dtypes.bfloat16
P = 128
C = 1024
NPAIR = 8   # head pairs
NKT = 8     # C / 128 contraction tiles
NW = 16     # window token blocks (2048 tokens)
NT = 8      # own token blocks (1024 tokens)
ND = 32     # DFF / 128 tiles
EPS = 1e-5

f32 = mybir.dt.float32
bf16 = mybir.dt.bfloat16
FT = mybir.ActivationFunctionType
ALU = mybir.AluOpType


def _build(flags):
    nc = bacc.Bacc("TRN2", target_bir_lowering=False, debug=False, num_devices=8)

    xkv = nc.dram_tensor("xkv", [2048, C], f32, kind="ExternalInput")
    wq = nc.dram_tensor("wq", [P, 8192], bf16, kind="ExternalInput")
    wk = nc.dram_tensor("wk", [P, 8192], bf16, kind="ExternalInput")
    wv = nc.dram_tensor("wv", [P, 8192], bf16, kind="ExternalInput")
    wo = nc.dram_tensor("wo", [P, 8192], bf16, kind="ExternalInput")
    w1 = nc.dram_tensor("w1", [P, 32768], bf16, kind="ExternalInput")
    w2 = nc.dram_tensor("w2", [P, 32768], bf16, kind="ExternalInput")
    masks = nc.dram_tensor("masks", [P, 2 * P], bf16, kind="ExternalInput")
    identd = nc.dram_tensor("identd", [P, P], bf16, kind="ExternalInput")
    qbias = nc.dram_tensor("qbias", [P, NPAIR], f32, kind="ExternalInput")
    kbias = nc.dram_tensor("kbias", [P, NPAIR], f32, kind="ExternalInput")
    b1p = nc.dram_tensor("b1p", [P, ND], f32, kind="ExternalInput")
    bo_row = nc.dram_tensor("bo_row", [P, C], f32, kind="ExternalInput")
    b2_row = nc.dram_tensor("b2_row", [P, C], f32, kind="ExternalInput")
    out = nc.dram_tensor("out", [1024, C], bf16, kind="ExternalOutput")

    with tile.TileContext(nc) as tc, ExitStack() as es:
        consts = es.enter_context(tc.tile_pool(name="consts", bufs=1))
        mk_sb = consts.tile([P, 2 * P], bf16, tag="mk")
        nc.sync.dma_start(out=mk_sb[:, :], in_=masks.ap()[:, :])
        ident_sb = consts.tile([P, P], bf16, tag="ident")
        nc.sync.dma_start(out=ident_sb[:, :], in_=identd.ap()[:, :])
        qb_sb = consts.tile([P, NPAIR], f32, tag="qb")
        nc.sync.dma_start(out=qb_sb[:, :], in_=qbias.ap()[:, :])
        kb_sb = consts.tile([P, NPAIR], f32, tag="kb")
        nc.sync.dma_start(out=kb_sb[:, :], in_=kbias.ap()[:, :])
        b1_sb = consts.tile([P, ND], f32, tag="b1")
        nc.sync.dma_start(out=b1_sb[:, :], in_=b1p.ap()[:, :])
        eps_sb = consts.tile([P, 1], f32, tag="eps")
        nc.vector.memset(eps_sb[:, :], EPS)
        if flags["bo"]:
            bo_sb = consts.tile([P, C], f32, tag="bo")
            nc.sync.dma_start(out=bo_sb[:, :], in_=bo_row.ap()[:, :])
        if flags["b2"]:
            b2_sb = consts.tile([P, C], f32, tag="b2")
            nc.sync.dma_start(out=b2_sb[:, :], in_=b2_row.ap()[:, :])

        # persistent activation storage
        qt_pool = es.enter_context(tc.tile_pool(name="qt", bufs=NPAIR))
        kt_pool = es.enter_context(tc.tile_pool(name="kt", bufs=NPAIR))
        v_pool = es.enter_context(tc.tile_pool(name="vv", bufs=NW))
        x_pool = es.enter_context(tc.tile_pool(name="xx", bufs=NT))
        o_pool = es.enter_context(tc.tile_pool(name="oo", bufs=4, side="right"))
        QT = [qt_pool.tile([P, 1024], bf16, tag="qt", name=f"qt{i}") for i in range(NPAIR)]
        KT = [kt_pool.tile([P, 2048], bf16, tag="kt", name=f"kt{i}") for i in range(NPAIR)]
        # V with interleaved ones columns: per pair 65+65 cols
        VO = [v_pool.tile([P, NPAIR * 130], bf16, tag="vv", name=f"vo{i}") for i in range(NW)]
        X = [x_pool.tile([P, C], bf16, tag="xx", name=f"xt{i}") for i in range(NT)]
        O = [o_pool.tile([P, C], bf16, tag="oo", name=f"ot{i}") for i in range(NT)]

        def ln_tile(src_ap, lnp, zpool):
            """LayerNorm a [128, C] fp32 tile -> bf16 z tile (g/b folded out)."""
            if isinstance(src_ap, tuple):
                xw = lnp.tile([P, C], f32, tag="xw", name="xw", bufs=4)
                nc.sync.dma_start(out=xw[:, :], in_=src_ap[0])
            else:
                xw = src_ap
            stats = lnp.tile([P, 2, 6], f32, tag="stats")
            nc.vector.bn_stats(out=stats[:, 0, :], in_=xw[:, 0:512])
            nc.vector.bn_stats(out=stats[:, 1, :], in_=xw[:, 512:1024])
            mv = lnp.tile([P, 2], f32, tag="mv")
            nc.vector.bn_aggr(out=mv[:, :], in_=stats[:, :, :])
            rsig = lnp.tile([P, 1], f32, tag="rsig")
            nc.scalar.activation(rsig[:, :], mv[:, 1:2], FT.Sqrt,
                                 bias=eps_sb[:, :], scale=1.0)
            nc.vector.reciprocal(rsig[:, :], rsig[:, :])
            z = zpool.tile([P, C], bf16, tag="z")
            nc.vector.tensor_scalar(z[:, :], xw[:, :], mv[:, 0:1], rsig[:, :],
                                    ALU.subtract, ALU.mult)
            return z

        # ---------------- Phase A: LN1, hT, V projection ----------------
        ht_es = ExitStack()
        htp = ht_es.enter_context(tc.tile_pool(name="ht", bufs=1))
        HT = [htp.tile([P, 2048], bf16, tag=f"ht{i}", name=f"ht{i}") for i in range(NKT)]
        with tc.tile_pool(name="ln1", bufs=3) as lnp, \
             tc.tile_pool(name="z1", bufs=3) as zpool, \
             tc.tile_pool(name="tps1", bufs=2, space="PSUM") as tps1, \
             tc.tile_pool(name="wvp", bufs=NKT) as wv_pool, \
             tc.tile_pool(name="wka", bufs=3) as wka_pool, \
             tc.tile_pool(name="kqa", bufs=2, space="PSUM") as kqa, \
             tc.tile_pool(name="qkvps", bufs=2, space="PSUM") as qkvps:
            WV = [wv_pool.tile([P, 1024], bf16, tag="wv", name=f"wvt{i}") for i in range(NKT)]
            xw_pre = []
            for w in range(3):
                xf = lnp.tile([P, C], f32, tag="xw", name="xwp", bufs=4)
                nc.sync.dma_start(out=xf[:, :],
                                  in_=xkv.ap()[w * P:(w + 1) * P, :])
                xw_pre.append(xf)
            for kt in range(NKT):
                nc.sync.dma_start(out=WV[kt][:, :],
                                  in_=wv.ap()[:, kt * 1024:(kt + 1) * 1024])

            def k_step(wh, pr):
                wk_sb = wka_pool.tile([P, 1024], bf16, tag="wka", name="wka")
                nc.sync.dma_start(out=wk_sb[:, :],
                                  in_=wk.ap()[:, pr * 1024:(pr + 1) * 1024])
                pk = kqa.tile([P, 512], f32, tag="kqa", name="pk")
                for kt in range(NKT):
                    nc.tensor.matmul(
                        pk[:, :], wk_sb[:, kt * P:(kt + 1) * P],
                        HT[kt][:, wh * 512:(wh + 1) * 512],
                        start=(kt == 0), stop=(kt == NKT - 1))
                nc.vector.tensor_scalar_add(
                    KT[pr][:, wh * 512:(wh + 1) * 512], pk[:, :],
                    kb_sb[:, pr:pr + 1])

            for w in range(NW):
                if w < 3:
                    z = ln_tile(xw_pre[w], lnp, zpool)
                else:
                    z = ln_tile((xkv.ap()[w * P:(w + 1) * P, :],), lnp, zpool)
                for c in range(NKT):
                    tp = tps1.tile([P, P], bf16, tag="tp")
                    nc.tensor.transpose(tp[:, :], z[:, c * P:(c + 1) * P],
                                        ident_sb[:, :])
                    nc.scalar.copy(
                        out=HT[c][:, w * P:(w + 1) * P], in_=tp[:, :])
                pv = qkvps.tile([P, 1024], f32, tag="qkvps")
                for kt in range(NKT):
                    for hf in range(2):
                        nc.tensor.matmul(
                            pv[:, hf * 512:(hf + 1) * 512],
                            HT[kt][:, w * P:(w + 1) * P],
                            WV[kt][:, hf * 512:(hf + 1) * 512],
                            start=(kt == 0), stop=(kt == NKT - 1))
                vdst = VO[w][:, :].rearrange("p (pr hi dd) -> p pr hi dd",
                                             pr=NPAIR, hi=2)[:, :, :, 0:64]
                vsrc = pv[:, :].rearrange("p (pr hi dd) -> p pr hi dd",
                                          pr=NPAIR, hi=2)
                nc.scalar.copy(out=vdst, in_=vsrc)
                ones = VO[w][:, :].rearrange("p (pr hi dd) -> p pr hi dd",
                                             pr=NPAIR, hi=2)[:, :, :, 64:65]
                nc.vector.memset(ones, 1.0)
                if 3 <= w <= 14:
                    wh = (w - 3) // 4
                    for pr in (2 * ((w - 3) % 4), 2 * ((w - 3) % 4) + 1):
                        k_step(wh, pr)
                elif w == 15:
                    for pr in range(NPAIR):
                        k_step(3, pr)
            # prefetch own-token residual rows (own = odd permuted blocks)
            for t in range(NT):
                xf = lnp.tile([P, C], f32, tag="xw", name="xf", bufs=4)
                nc.sync.dma_start(
                    out=xf[:, :],
                    in_=xkv.ap()[(2 * t + 1) * P:(2 * t + 2) * P, :])
                nc.vector.tensor_copy(out=X[t][:, :], in_=xf[:, :])

        # ---------------- attention (chunk c, head pair pr) ----------------
        attn_es = ExitStack()
        sps = attn_es.enter_context(tc.tile_pool(name="sps", bufs=2, space="PSUM"))
        ops_pool = attn_es.enter_context(tc.tile_pool(name="ops", bufs=2, space="PSUM"))
        ep_pool = attn_es.enter_context(tc.tile_pool(name="epp", bufs=3, side="right"))
        sal_pool = attn_es.enter_context(tc.tile_pool(name="sal", bufs=4, side="right"))
        wo_pool = attn_es.enter_context(tc.tile_pool(name="wos", bufs=1, side="right"))
        wo_sb = wo_pool.tile([P, 8192], bf16, tag="wo")
        nc.sync.dma_start(out=wo_sb[:, :], in_=wo.ap()[:, :])

        def attn_pr_steps(c, pr):
            """List of closures: full attention of chunk c for head pair pr."""
            state = {}

            def alloc():
                state[0] = ops_pool.tile([P, 260], f32, tag="ops", name="opsA")
                state[1] = ops_pool.tile([P, 260], f32, tag="ops", name="opsB")

            def do_j(j):
                q0 = max(0, (j // 2 - 4 * c)) * P
                qlen = 512 - q0
                sp = sps.tile([P, 1024], f32, tag="sps")
                for hi in range(2):
                    nc.tensor.matmul(
                        sp[:, hi * 512:hi * 512 + qlen],
                        KT[pr][hi * 64:(hi + 1) * 64, j * P:(j + 1) * P],
                        QT[pr][hi * 64:(hi + 1) * 64, c * 512 + q0:(c + 1) * 512],
                        start=True, stop=True)
                ep = ep_pool.tile([P, 1024], bf16, tag="ep")
                spv = sp[:, :].rearrange("p (hi q) -> p hi q", hi=2)[:, :, 0:qlen]
                epv = ep[:, 0:2 * qlen].rearrange("p (hi q) -> p hi q", hi=2)
                nc.scalar.activation(epv, spv, FT.Exp)
                t_d = j // 2
                if t_d >= 4 * c:
                    off = (t_d - 4 * c) * P - q0
                    mcol = (j % 2) * P
                    for hi in range(2):
                        sl = ep[:, hi * qlen + off:hi * qlen + off + P]
                        nc.vector.tensor_mul(sl, sl, mk_sb[:, mcol:mcol + P])
                for t in range(max(4 * c, j // 2), 4 * c + 4):
                    tl = t - 4 * c
                    gt = state[tl // 2]
                    gc = (tl % 2) * 130
                    off = tl * P - q0
                    # one start / one stop per psum bank (accumulation group)
                    for hi in range(2):
                        nc.tensor.matmul(
                            gt[:, gc + hi * 65:gc + (hi + 1) * 65],
                            ep[:, hi * qlen + off:hi * qlen + off + P],
                            VO[j][:, pr * 130 + hi * 65:pr * 130 + (hi + 1) * 65],
                            start=(j == 0 and hi == 0 and tl % 2 == 0),
                            stop=(hi == 1 and
                                  ((j == 8 * c + 3 and tl == 1) or
                                   (j == 8 * c + 7 and tl == 3))))

            def norm():
                for tl in range(4):
                    t = 4 * c + tl
                    gt = state[tl // 2]
                    gc = (tl % 2) * 130
                    rs = sal_pool.tile([P, 2], f32, tag="rs")
                    rsrc = gt[:, gc:gc + 130].rearrange(
                        "p (hi d) -> p hi d", hi=2)[:, :, 64:65]
                    nc.vector.reciprocal(rs[:, :], rsrc)
                    for hi in range(2):
                        nc.vector.tensor_scalar_mul(
                            O[t][:, pr * P + hi * 64:pr * P + hi * 64 + 64],
                            gt[:, gc + hi * 65:gc + hi * 65 + 64],
                            rs[:, hi:hi + 1])

            steps = [alloc]
            steps += [(lambda j=j: do_j(j)) for j in range(8 * c + 8)]
            steps.append(norm)
            return steps

        # ---- Phase B: Q projection per pair + attention chunk 0 ----
        with tc.tile_pool(name="wqkv", bufs=2) as wqkv_pool, \
             tc.tile_pool(name="kqps", bufs=2, space="PSUM") as kqps:
            for pr in range(NPAIR):
                wq_sb = wqkv_pool.tile([P, 1024], bf16, tag="wqk")
                nc.sync.dma_start(out=wq_sb[:, :],
                                  in_=wq.ap()[:, pr * 1024:(pr + 1) * 1024])
                for qh in range(2):
                    pq = kqps.tile([P, 512], f32, tag="kq")
                    for kt in range(NKT):
                        qrhs = HT[kt][:, :].rearrange(
                            "p (t par d) -> p t par d", t=8, par=2)[
                                :, qh * 4:(qh + 1) * 4, 1:2, :]
                        nc.tensor.matmul(
                            pq[:, :], wq_sb[:, kt * P:(kt + 1) * P],
                            qrhs, start=(kt == 0), stop=(kt == NKT - 1))
                    nc.vector.tensor_scalar(
                        QT[pr][:, qh * 512:(qh + 1) * 512], pq[:, :],
                        qb_sb[:, pr:pr + 1], 0.125, ALU.add, ALU.mult)
                for s in attn_pr_steps(0, pr):
                    s()
        ht_es.close()

        # ---- shared FFN/Wo/transpose psum pool + weight streams ----
        fps = attn_es.enter_context(tc.tile_pool(name="fps", bufs=2, space="PSUM"))
        ot_pool = attn_es.enter_context(tc.tile_pool(name="otp", bufs=2))
        ln2p = attn_es.enter_context(tc.tile_pool(name="ln2", bufs=2))
        z2pool = attn_es.enter_context(tc.tile_pool(name="z2", bufs=2))
        h2t_pool = attn_es.enter_context(tc.tile_pool(name="h2t", bufs=1))
        ut_pool = attn_es.enter_context(tc.tile_pool(name="ut", bufs=ND))
        w1_pool = attn_es.enter_context(tc.tile_pool(name="w1s", bufs=3))
        w2_pool = attn_es.enter_context(tc.tile_pool(name="w2s", bufs=4))
        H2T = [h2t_pool.tile([P, 1024], bf16, tag=f"h2t{i}", name=f"h2t{i}")
               for i in range(NKT)]
        UT = [ut_pool.tile([P, 512], bf16, tag="ut", name=f"ut{i}") for i in range(ND)]

        def wo_t(t):
            """O[t] -> OT -> Wo -> X[t] residual (sps-pool psum)."""
            ot = ot_pool.tile([P, 1024], bf16, tag="ot")
            for kt in range(NKT):
                tp = sps.tile([P, 512], bf16, tag="sps", name="tpo")
                nc.tensor.transpose(tp[:, 0:P], O[t][:, kt * P:(kt + 1) * P],
                                    ident_sb[:, :])
                nc.scalar.copy(out=ot[:, kt * P:(kt + 1) * P],
                               in_=tp[:, 0:P])
            for hf in range(2):
                pw = sps.tile([P, 512], f32, tag="sps", name="pw")
                for kt in range(NKT):
                    nc.tensor.matmul(
                        pw[:, :], ot[:, kt * P:(kt + 1) * P],
                        wo_sb[:, kt * 1024 + hf * 512:kt * 1024 + (hf + 1) * 512],
                        start=(kt == 0), stop=(kt == NKT - 1))
                xsl = X[t][:, hf * 512:(hf + 1) * 512]
                nc.vector.tensor_add(xsl, pw[:, :], xsl)
                if flags["bo"]:
                    nc.vector.tensor_add(xsl, xsl, bo_sb[:, hf * 512:(hf + 1) * 512])

        def ln2_t(t):
            """X[t] -> LN2 -> z2 -> H2T columns (sps-pool psum)."""
            z2 = ln_tile(X[t], ln2p, z2pool)
            c, tl = t // 4, t % 4
            for kt in range(NKT):
                tp = sps.tile([P, 512], bf16, tag="sps", name="tpz")
                nc.tensor.transpose(tp[:, 0:P], z2[:, kt * P:(kt + 1) * P],
                                    ident_sb[:, :])
                nc.scalar.copy(
                    out=H2T[kt][:, c * 512 + tl * P:c * 512 + (tl + 1) * P],
                    in_=tp[:, 0:P])

        def ffn_steps(tch, chs=(0, 1), do_w1=True, dual_q=False):
            """Closures for FFN on token chunk tch (512 tokens)."""
            steps = []
            state = {}

            def w1_d(d):
                w1_sb = w1_pool.tile([P, 1024], bf16, tag="w1")
                nc.sync.dma_start(out=w1_sb[:, :],
                                  in_=w1.ap()[:, d * 1024:(d + 1) * 1024])
                pu = fps.tile([P, 512], f32, tag="fps")
                for kt in range(NKT):
                    nc.tensor.matmul(
                        pu[:, :], w1_sb[:, kt * P:(kt + 1) * P],
                        H2T[kt][:, tch * 512:(tch + 1) * 512],
                        start=(kt == 0), stop=(kt == NKT - 1))
                nc.vector.tensor_scalar(UT[d][:, :], pu[:, :],
                                        b1_sb[:, d:d + 1], 0.0,
                                        ALU.add, ALU.max)

            def w2_start():
                state["y0"] = fps.tile([P, 512], f32, tag="fps", name="y0")
                state["y1"] = fps.tile([P, 512], f32, tag="fps", name="y1")

            def w2_dp(ch, tp_, dp):
                w2_sb = w2_pool.tile([P, 2, 512], bf16, tag="w2")
                qeng = nc.scalar if (dual_q and dp % 2) else nc.sync
                qeng.dma_start(
                    out=w2_sb[:, :, :],
                    in_=w2.ap()[:, :].rearrange("p (d n) -> p d n", d=ND)[
                        :, 2 * dp:2 * dp + 2, ch * 512:(ch + 1) * 512])
                for dk in range(2):
                    d = 2 * dp + dk
                    for k in range(2):
                        tt = tp_ * 2 + k
                        nc.tensor.matmul(
                            state[f"y{k}"][:, :],
                            UT[d][:, tt * P:(tt + 1) * P], w2_sb[:, dk:dk + 1, :],
                            start=(d == 0), stop=(d == ND - 1))

            def w2_end(ch, tp_):
                for k in range(2):
                    t = tch * 4 + tp_ * 2 + k
                    xsl = X[t][:, ch * 512:(ch + 1) * 512]
                    nc.vector.tensor_add(xsl, state[f"y{k}"][:, :], xsl)
                    if flags["b2"]:
                        nc.vector.tensor_add(
                            xsl, xsl, b2_sb[:, ch * 512:(ch + 1) * 512])

            if do_w1:
                for d in range(ND):
                    steps.append(lambda d=d: w1_d(d))
            for ch in chs:
                for tp_ in range(2):
                    steps.append(w2_start)
                    for dp in range(ND // 2):
                        steps.append(lambda ch=ch, tp_=tp_, dp=dp: w2_dp(ch, tp_, dp))
                    steps.append(lambda ch=ch, tp_=tp_: w2_end(ch, tp_))
            return steps

        # ---- Phase C: Wo + LN2 for chunk 0, software-pipelined ----
        wo_t(0)
        wo_t(1)
        ln2_t(0)
        wo_t(2)
        ln2_t(1)
        wo_t(3)
        ln2_t(2)
        ln2_t(3)

        # ---- Phase D: FFN(tch0, W1+W2ch0) interleaved with attn chunk 1 ----
        f_steps = ffn_steps(0, chs=(0,))
        a_steps = []
        for pr in range(NPAIR):
            a_steps += attn_pr_steps(1, pr)
        fi = ai = 0
        while fi < len(f_steps) or ai < len(a_steps):
            if fi < len(f_steps):
                f_steps[fi]()
                fi += 1
            if ai < len(a_steps):
                a_steps[ai]()
                ai += 1
            if ai < len(a_steps):
                a_steps[ai]()
                ai += 1

        # ---- Phase E: Wo + LN2 chunk 1 (sps psum) || FFN0 W2 ch1 (fps) ----
        e_steps = [lambda: wo_t(4), lambda: wo_t(5), lambda: ln2_t(4),
                   lambda: wo_t(6), lambda: ln2_t(5), lambda: wo_t(7),
                   lambda: ln2_t(6), lambda: ln2_t(7)]
        f2_steps = ffn_steps(0, chs=(1,), do_w1=False, dual_q=True)
        fi = ei = 0
        while fi < len(f2_steps) or ei < len(e_steps):
            for _ in range(5):
                if fi < len(f2_steps):
                    f2_steps[fi]()
                    fi += 1
            if ei < len(e_steps):
                e_steps[ei]()
                ei += 1
        for t in range(4):
            nc.sync.dma_start(out=out.ap()[t * P:(t + 1) * P, :], in_=X[t][:, :])

        # ---- Phase F: FFN(tch1): W1 pipelined with W2ch0, then W2ch1 ----
        tch = 1

        def f_w1(d):
            w1_sb = w1_pool.tile([P, 1024], bf16, tag="w1")
            nc.sync.dma_start(out=w1_sb[:, :],
                              in_=w1.ap()[:, d * 1024:(d + 1) * 1024])
            pu = fps.tile([P, 512], f32, tag="fps")
            for kt in range(NKT):
                nc.tensor.matmul(
                    pu[:, :], w1_sb[:, kt * P:(kt + 1) * P],
                    H2T[kt][:, tch * 512:(tch + 1) * 512],
                    start=(kt == 0), stop=(kt == NKT - 1))
            nc.scalar.activation(UT[d][:, :], pu[:, :], FT.Relu,
                                 bias=b1_sb[:, d:d + 1], scale=1.0)

        def f_w2grp(ch, d, ya, yb, w2_sb, dk, dq=False):
            for k in range(2):
                for tp_ in range(2):
                    tt = tp_ * 2 + k
                    y = (ya, yb)[tp_]
                    nc.tensor.matmul(
                        y[:, k * 512:(k + 1) * 512],
                        UT[d][:, tt * P:(tt + 1) * P], w2_sb[:, dk:dk + 1, :],
                        start=(d == 0), stop=(d == ND - 1))

        def f_w2dma(ch, dp, dq=False):
            w2_sb = w2_pool.tile([P, 2, 512], bf16, tag="w2")
            qeng = nc.scalar if (dq and dp % 2) else nc.sync
            qeng.dma_start(
                out=w2_sb[:, :, :],
                in_=w2.ap()[:, :].rearrange("p (d n) -> p d n", d=ND)[
                    :, 2 * dp:2 * dp + 2, ch * 512:(ch + 1) * 512])
            return w2_sb

        def f_w2end(ch, ya, yb):
            for tp_ in range(2):
                for k in range(2):
                    t = tch * 4 + tp_ * 2 + k
                    xsl = X[t][:, ch * 512:(ch + 1) * 512]
                    nc.vector.tensor_add(xsl, (ya, yb)[tp_][:, k * 512:(k + 1) * 512], xsl)
                    if flags["b2"]:
                        nc.vector.tensor_add(
                            xsl, xsl, b2_sb[:, ch * 512:(ch + 1) * 512])

        ya = sps.tile([P, 1024], f32, tag="sps", name="ya")
        yb = sps.tile([P, 1024], f32, tag="sps", name="yb")
        f_w1(0)
        f_w1(1)
        w2t = {}
        for d in range(ND):
            if d % 2 == 0:
                w2t[d // 2] = f_w2dma(0, d // 2)
            f_w2grp(0, d, ya, yb, w2t[d // 2], d % 2)
            if d + 2 < ND:
                f_w1(d + 2)
        f_w2end(0, ya, yb)
        ya2 = sps.tile([P, 1024], f32, tag="sps", name="ya2")
        yb2 = sps.tile([P, 1024], f32, tag="sps", name="yb2")
        for dp in range(ND // 2):
            w2_sb = f_w2dma(1, dp, dq=True)
            for dk in range(2):
                f_w2grp(1, 2 * dp + dk, ya2, yb2, w2_sb, dk)
        f_w2end(1, ya2, yb2)
        for t in range(4, 8):
            nc.sync.dma_start(out=out.ap()[t * P:(t + 1) * P, :], in_=X[t][:, :])
        attn_es.close()

    nc.compile()
    return nc


_CACHE = {}


def _prep(inputs):
    """Host-side preprocessing: fold LN affine into weights, tile/cast, shard."""
    x = np.asarray(inputs["x"], np.float32)
    Wq = np.asarray(inputs["Wq"], np.float32)
    Wk = np.asarray(inputs["Wk"], np.float32)
    Wv = np.asarray(inputs["Wv"], np.float32)
    Wo = np.asarray(inputs["Wo"], np.float32)
    bo = np.asarray(inputs["bo"], np.float32)
    W1 = np.asarray(inputs["W1"], np.float32)
    b1 = np.asarray(inputs["b1"], np.float32)
    W2 = np.asarray(inputs["W2"], np.float32)
    b2 = np.asarray(inputs["b2"], np.float32)
    g1 = np.asarray(inputs["g1"], np.float32)
    be1 = np.asarray(inputs["be1"], np.float32)
    g2 = np.asarray(inputs["g2"], np.float32)
    be2 = np.asarray(inputs["be2"], np.float32)

    Wq_g = (Wq * g1[None, :, None]).astype(BF16)   # [16,1024,64]
    Wk_g = (Wk * g1[None, :, None]).astype(BF16)
    Wv_g = (Wv * g1[None, :, None]).astype(BF16)
    qb = np.einsum('c,hcd->hd', be1, Wq_g.astype(np.float32))  # [16,64]
    kb = np.einsum('c,hcd->hd', be1, Wk_g.astype(np.float32))
    vb = np.einsum('c,hcd->hd', be1, Wv_g.astype(np.float32))
    if np.abs(vb).max() > 0:
        raise NotImplementedError("nonzero folded V bias not supported")

    def lhsT_pack(wflat):  # [1024 c, 1024 m] -> [128, (pair, kt, 128)]
        return np.ascontiguousarray(
            wflat.reshape(8, 128, 8, 128).transpose(1, 2, 0, 3).reshape(128, 8192))

    def rhs_pack(wflat):   # [1024 k, 1024 n] -> [128, (kt, 1024)]
        return np.ascontiguousarray(
            wflat.reshape(8, 128, 1024).transpose(1, 0, 2).reshape(128, 8192))

    wq_h = lhsT_pack(Wq_g.transpose(1, 0, 2).reshape(1024, 1024))
    wk_h = lhsT_pack(Wk_g.transpose(1, 0, 2).reshape(1024, 1024))
    wv_h = rhs_pack(Wv_g.transpose(1, 0, 2).reshape(1024, 1024))
    wo_h = rhs_pack(Wo.astype(BF16))
    W1_g = (W1 * g2[:, None]).astype(BF16)         # [1024, 4096]
    b1p = b1 + be2 @ W1_g.astype(np.float32)
    w1_h = np.ascontiguousarray(
        W1_g.reshape(8, 128, 32, 128).transpose(1, 2, 0, 3).reshape(128, 32768))
    w2_h = np.ascontiguousarray(
        W2.astype(BF16).reshape(32, 128, 1024).transpose(1, 0, 2).reshape(128, 32768))

    qb_t = np.zeros((128, 8), np.float32)
    kb_t = np.zeros((128, 8), np.float32)
    for pr in range(8):
        qb_t[0:64, pr] = qb[2 * pr]
        qb_t[64:128, pr] = qb[2 * pr + 1]
        kb_t[0:64, pr] = kb[2 * pr]
        kb_t[64:128, pr] = kb[2 * pr + 1]
    b1_t = np.ascontiguousarray(b1p.reshape(32, 128).T.astype(np.float32))
    bo_t = np.broadcast_to(bo, (128, 1024)).astype(np.float32).copy()
    b2_t = np.broadcast_to(b2, (128, 1024)).astype(np.float32).copy()

    triu = np.triu(np.ones((128, 128), np.float32))
    ident = np.eye(128, dtype=np.float32).astype(BF16)

    flags = {"bo": bool(np.abs(bo).max() > 0), "b2": bool(np.abs(b2).max() > 0)}

    shared = dict(wq=wq_h, wk=wk_h, wv=wv_h, wo=wo_h, w1=w1_h, w2=w2_h,
                  identd=ident, qbias=qb_t, kbias=kb_t,
                  b1p=b1_t, bo_row=bo_t, b2_row=b2_t)
    in_maps = []
    for core in range(8):
        b, par = core // 2, core % 2
        xb = x[b].reshape(16, 128, 1024)
        if par == 0:
            # swap even/odd blocks so own (even-global) blocks sit at odd slots
            perm = [i + 1 if i % 2 == 0 else i - 1 for i in range(16)]
            xw = np.ascontiguousarray(xb[perm].reshape(2048, 1024))
            m1 = np.zeros((128, 128), np.float32)
        else:
            xw = np.ascontiguousarray(xb.reshape(2048, 1024))
            m1 = np.ones((128, 128), np.float32)
        mk = np.concatenate([m1, triu], axis=1).astype(BF16)
        in_maps.append({"xkv": xw, "masks": mk, **shared})
    return in_maps, flags


def _get_nc(flags):
    key = tuple(sorted(flags.items()))
    if key not in _CACHE:
        _CACHE[key] = _build(flags)
    return _CACHE[key]


def run(inputs, **kw):
    in_maps, flags = _prep(inputs)
    nc = _get_nc(flags)
    res = run_bass_kernel_spmd(nc, in_maps, core_ids=list(range(8)), **kw)
    x = np.asarray(inputs["x"], np.float32)
    outf = np.zeros_like(x)
    for core in range(8):
        b, par = core // 2, core % 2
        r = np.asarray(res.results[core]["out"], np.float32)
        for t in range(8):
            g = 2 * t + par
            outf[b, g * 128:(g + 1) * 128] = r[t * 128:(t + 1) * 128]
    return outf, res


def kernel(**inputs):
    outf, _ = run(inputs)
    return outf


# revision 17
# speedup vs baseline: 1.2165x; 1.0061x over previous
"""Trainium2 Bass kernel for a dense transformer decoder block (B=4, T=2048,
C=1024, 16 heads x 64, DFF=4096), SPMD across 8 NeuronCores.

Sharding: core pair (2b, 2b+1) shares batch element b. Token blocks (128) are
interleaved between the pair so both cores see near-equal causal windows:
core par owns global blocks {2t+par}. The host permutes each core's token
order so OWN blocks always sit at odd positions 2t+1 -> one uniform SPMD
program. Causal masks for the last two window blocks of each query block are
per-core DATA (m1: zeros for par=0 / ones for par=1; m2: triu for both).

Pipeline: LN1+V -> per-pair K/Q || attention(chunk0) -> Wo/LN2(c0) ->
FFN(tch0) || attention(chunk1) -> Wo/LN2(c1) -> FFN(tch1). The softmax exp
(scalar engine) hides behind FFN/projection matmuls on the tensor engine.

All matmuls bf16 (fp32 PSUM); LN stats, softmax normalization, residuals fp32.
LN gamma/beta folded into adjacent weights on host.
"""

import os
from contextlib import ExitStack

os.environ.setdefault("MYCRO_LOCAL_CACHE", "1")

import numpy as np
import ml_dtypes

import concourse.bacc as bacc
import concourse.bass as bass
import concourse.mybir as mybir
import concourse.tile as tile
from concourse.bass_utils import run_bass_kernel_spmd

BF16 = ml_dtypes.bfloat16
P = 128
C = 1024
NPAIR = 8   # head pairs
NKT = 8     # C / 128 contraction tiles
NW = 16     # window token blocks (2048 tokens)
NT = 8      # own token blocks (1024 tokens)
ND = 32     # DFF / 128 tiles
EPS = 1e-5

f32 = mybir.dt.float32
bf16 = mybir.dt.bfloat16
FT = mybir.ActivationFunctionType
ALU = mybir.AluOpType


def _build(flags):
    nc = bacc.Bacc("TRN2", target_bir_lowering=False, debug=False, num_devices=8)

    xkv = nc.dram_tensor("xkv", [2048, C], f32, kind="ExternalInput")
    wq = nc.dram_tensor("wq", [P, 8192], bf16, kind="ExternalInput")
    wk = nc.dram_tensor("wk", [P, 8192], bf16, kind="ExternalInput")
    wv = nc.dram_tensor("wv", [P, 8192], bf16, kind="ExternalInput")
    wo = nc.dram_tensor("wo", [P, 8192], bf16, kind="ExternalInput")
    w1 = nc.dram_tensor("w1", [P, 32768], bf16, kind="ExternalInput")
    w2 = nc.dram_tensor("w2", [P, 32768], bf16, kind="ExternalInput")
    masks = nc.dram_tensor("masks", [P, 2 * P], bf16, kind="ExternalInput")
    identd = nc.dram_tensor("identd", [P, P], bf16, kind="ExternalInput")
    qbias = nc.dram_tensor("qbias", [P, NPAIR], f32, kind="ExternalInput")
    kbias = nc.dram_tensor("kbias", [P, NPAIR], f32, kind="ExternalInput")
    b1p = nc.dram_tensor("b1p", [P, ND], f32, kind="ExternalInput")
    bo_row = nc.dram_tensor("bo_row", [P, C], f32, kind="ExternalInput")
    b2_row = nc.dram_tensor("b2_row", [P, C], f32, kind="ExternalInput")
    out = nc.dram_tensor("out", [1024, C], bf16, kind="ExternalOutput")

    with tile.TileContext(nc) as tc, ExitStack() as es:
        consts = es.enter_context(tc.tile_pool(name="consts", bufs=1))
        mk_sb = consts.tile([P, 2 * P], bf16, tag="mk")
        nc.sync.dma_start(out=mk_sb[:, :], in_=masks.ap()[:, :])
        ident_sb = consts.tile([P, P], bf16, tag="ident")
        nc.sync.dma_start(out=ident_sb[:, :], in_=identd.ap()[:, :])
        qb_sb = consts.tile([P, NPAIR], f32, tag="qb")
        nc.sync.dma_start(out=qb_sb[:, :], in_=qbias.ap()[:, :])
        kb_sb = consts.tile([P, NPAIR], f32, tag="kb")
        nc.sync.dma_start(out=kb_sb[:, :], in_=kbias.ap()[:, :])
        b1_sb = consts.tile([P, ND], f32, tag="b1")
        nc.sync.dma_start(out=b1_sb[:, :], in_=b1p.ap()[:, :])
        eps_sb = consts.tile([P, 1], f32, tag="eps")
        nc.vector.memset(eps_sb[:, :], EPS)
        if flags["bo"]:
            bo_sb = consts.tile([P, C], f32, tag="bo")
            nc.sync.dma_start(out=bo_sb[:, :], in_=bo_row.ap()[:, :])
        if flags["b2"]:
            b2_sb = consts.tile([P, C], f32, tag="b2")
            nc.sync.dma_start(out=b2_sb[:, :], in_=b2_row.ap()[:, :])

        # persistent activation storage
        qt_pool = es.enter_context(tc.tile_pool(name="qt", bufs=NPAIR))
        kt_pool = es.enter_context(tc.tile_pool(name="kt", bufs=NPAIR))
        v_pool = es.enter_context(tc.tile_pool(name="vv", bufs=NW))
        x_pool = es.enter_context(tc.tile_pool(name="xx", bufs=NT))
        o_pool = es.enter_context(tc.tile_pool(name="oo", bufs=4, side="right"))
        QT = [qt_pool.tile([P, 1024], bf16, tag="qt", name=f"qt{i}") for i in range(NPAIR)]
        KT = [kt_pool.tile([P, 2048], bf16, tag="kt", name=f"kt{i}") for i in range(NPAIR)]
        # V with interleaved ones columns: per pair 65+65 cols
        VO = [v_pool.tile([P, NPAIR * 130], bf16, tag="vv", name=f"vo{i}") for i in range(NW)]
        X = [x_pool.tile([P, C], bf16, tag="xx", name=f"xt{i}") for i in range(NT)]
        O = [o_pool.tile([P, C], bf16, tag="oo", name=f"ot{i}") for i in range(NT)]

        def ln_tile(src_ap, lnp, zpool):
            """LayerNorm a [128, C] fp32 tile -> bf16 z tile (g/b folded out)."""
            if isinstance(src_ap, tuple):
                xw = lnp.tile([P, C], f32, tag="xw", name="xw", bufs=4)
                nc.sync.dma_start(out=xw[:, :], in_=src_ap[0])
            else:
                xw = src_ap
            stats = lnp.tile([P, 2, 6], f32, tag="stats")
            nc.vector.bn_stats(out=stats[:, 0, :], in_=xw[:, 0:512])
            nc.vector.bn_stats(out=stats[:, 1, :], in_=xw[:, 512:1024])
            mv = lnp.tile([P, 2], f32, tag="mv")
            nc.vector.bn_aggr(out=mv[:, :], in_=stats[:, :, :])
            rsig = lnp.tile([P, 1], f32, tag="rsig")
            nc.scalar.activation(rsig[:, :], mv[:, 1:2], FT.Sqrt,
                                 bias=eps_sb[:, :], scale=1.0)
            nc.vector.reciprocal(rsig[:, :], rsig[:, :])
            z = zpool.tile([P, C], bf16, tag="z")
            nc.vector.tensor_scalar(z[:, :], xw[:, :], mv[:, 0:1], rsig[:, :],
                                    ALU.subtract, ALU.mult)
            return z

        # ---------------- Phase A: LN1, hT, V projection ----------------
        ht_es = ExitStack()
        htp = ht_es.enter_context(tc.tile_pool(name="ht", bufs=1))
        HT = [htp.tile([P, 2048], bf16, tag=f"ht{i}", name=f"ht{i}") for i in range(NKT)]
        with tc.tile_pool(name="ln1", bufs=3) as lnp, \
             tc.tile_pool(name="z1", bufs=3) as zpool, \
             tc.tile_pool(name="tps1", bufs=2, space="PSUM") as tps1, \
             tc.tile_pool(name="wvp", bufs=NKT) as wv_pool, \
             tc.tile_pool(name="wka", bufs=3) as wka_pool, \
             tc.tile_pool(name="kqa", bufs=2, space="PSUM") as kqa, \
             tc.tile_pool(name="qkvps", bufs=2, space="PSUM") as qkvps:
            WV = [wv_pool.tile([P, 1024], bf16, tag="wv", name=f"wvt{i}") for i in range(NKT)]
            xw_pre = []
            for w in range(3):
                xf = lnp.tile([P, C], f32, tag="xw", name="xwp", bufs=4)
                nc.sync.dma_start(out=xf[:, :],
                                  in_=xkv.ap()[w * P:(w + 1) * P, :])
                xw_pre.append(xf)
            for kt in range(NKT):
                nc.sync.dma_start(out=WV[kt][:, :],
                                  in_=wv.ap()[:, kt * 1024:(kt + 1) * 1024])

            def k_step(wh, pr):
                wk_sb = wka_pool.tile([P, 1024], bf16, tag="wka", name="wka")
                nc.sync.dma_start(out=wk_sb[:, :],
                                  in_=wk.ap()[:, pr * 1024:(pr + 1) * 1024])
                pk = kqa.tile([P, 512], f32, tag="kqa", name="pk")
                for kt in range(NKT):
                    nc.tensor.matmul(
                        pk[:, :], wk_sb[:, kt * P:(kt + 1) * P],
                        HT[kt][:, wh * 512:(wh + 1) * 512],
                        start=(kt == 0), stop=(kt == NKT - 1))
                nc.vector.tensor_scalar_add(
                    KT[pr][:, wh * 512:(wh + 1) * 512], pk[:, :],
                    kb_sb[:, pr:pr + 1])

            for w in range(NW):
                if w < 3:
                    z = ln_tile(xw_pre[w], lnp, zpool)
                else:
                    z = ln_tile((xkv.ap()[w * P:(w + 1) * P, :],), lnp, zpool)
                for c in range(NKT):
                    tp = tps1.tile([P, P], bf16, tag="tp")
                    nc.tensor.transpose(tp[:, :], z[:, c * P:(c + 1) * P],
                                        ident_sb[:, :])
                    nc.scalar.copy(
                        out=HT[c][:, w * P:(w + 1) * P], in_=tp[:, :])
                pv = qkvps.tile([P, 1024], f32, tag="qkvps")
                for kt in range(NKT):
                    for hf in range(2):
                        nc.tensor.matmul(
                            pv[:, hf * 512:(hf + 1) * 512],
                            HT[kt][:, w * P:(w + 1) * P],
                            WV[kt][:, hf * 512:(hf + 1) * 512],
                            start=(kt == 0), stop=(kt == NKT - 1))
                vdst = VO[w][:, :].rearrange("p (pr hi dd) -> p pr hi dd",
                                             pr=NPAIR, hi=2)[:, :, :, 0:64]
                vsrc = pv[:, :].rearrange("p (pr hi dd) -> p pr hi dd",
                                          pr=NPAIR, hi=2)
                nc.scalar.copy(out=vdst, in_=vsrc)
                ones = VO[w][:, :].rearrange("p (pr hi dd) -> p pr hi dd",
                                             pr=NPAIR, hi=2)[:, :, :, 64:65]
                nc.vector.memset(ones, 1.0)
                if 3 <= w <= 14:
                    wh = (w - 3) // 4
                    for pr in (2 * ((w - 3) % 4), 2 * ((w - 3) % 4) + 1):
                        k_step(wh, pr)
                elif w == 15:
                    for pr in range(NPAIR):
                        k_step(3, pr)
            # prefetch own-token residual rows (own = odd permuted blocks)
            for t in range(NT):
                xf = lnp.tile([P, C], f32, tag="xw", name="xf", bufs=4)
                nc.sync.dma_start(
                    out=xf[:, :],
                    in_=xkv.ap()[(2 * t + 1) * P:(2 * t + 2) * P, :])
                nc.vector.tensor_copy(out=X[t][:, :], in_=xf[:, :])

        # ---------------- attention (chunk c, head pair pr) ----------------
        attn_es = ExitStack()
        sps = attn_es.enter_context(tc.tile_pool(name="sps", bufs=2, space="PSUM"))
        ops_pool = attn_es.enter_context(tc.tile_pool(name="ops", bufs=2, space="PSUM"))
        ep_pool = attn_es.enter_context(tc.tile_pool(name="epp", bufs=4, side="right"))
        sal_pool = attn_es.enter_context(tc.tile_pool(name="sal", bufs=4, side="right"))
        wo_pool = attn_es.enter_context(tc.tile_pool(name="wos", bufs=1, side="right"))
        wo_sb = wo_pool.tile([P, 8192], bf16, tag="wo")
        nc.sync.dma_start(out=wo_sb[:, :], in_=wo.ap()[:, :])

        def attn_pr_steps(c, pr):
            """List of closures: full attention of chunk c for head pair pr."""
            state = {}

            def alloc():
                state[0] = ops_pool.tile([P, 260], f32, tag="ops", name="opsA")
                state[1] = ops_pool.tile([P, 260], f32, tag="ops", name="opsB")

            def do_j(j):
                q0 = max(0, (j // 2 - 4 * c)) * P
                qlen = 512 - q0
                sp = sps.tile([P, 1024], f32, tag="sps")
                for hi in range(2):
                    nc.tensor.matmul(
                        sp[:, hi * 512:hi * 512 + qlen],
                        KT[pr][hi * 64:(hi + 1) * 64, j * P:(j + 1) * P],
                        QT[pr][hi * 64:(hi + 1) * 64, c * 512 + q0:(c + 1) * 512],
                        start=True, stop=True)
                ep = ep_pool.tile([P, 1024], bf16, tag="ep")
                spv = sp[:, :].rearrange("p (hi q) -> p hi q", hi=2)[:, :, 0:qlen]
                epv = ep[:, 0:2 * qlen].rearrange("p (hi q) -> p hi q", hi=2)
                nc.scalar.activation(epv, spv, FT.Exp)
                t_d = j // 2
                if t_d >= 4 * c:
                    off = (t_d - 4 * c) * P - q0
                    mcol = (j % 2) * P
                    for hi in range(2):
                        sl = ep[:, hi * qlen + off:hi * qlen + off + P]
                        nc.vector.tensor_mul(sl, sl, mk_sb[:, mcol:mcol + P])
                for t in range(max(4 * c, j // 2), 4 * c + 4):
                    tl = t - 4 * c
                    gt = state[tl // 2]
                    gc = (tl % 2) * 130
                    off = tl * P - q0
                    # one start / one stop per psum bank (accumulation group)
                    for hi in range(2):
                        nc.tensor.matmul(
                            gt[:, gc + hi * 65:gc + (hi + 1) * 65],
                            ep[:, hi * qlen + off:hi * qlen + off + P],
                            VO[j][:, pr * 130 + hi * 65:pr * 130 + (hi + 1) * 65],
                            start=(j == 0 and hi == 0 and tl % 2 == 0),
                            stop=(hi == 1 and
                                  ((j == 8 * c + 3 and tl == 1) or
                                   (j == 8 * c + 7 and tl == 3))))

            def norm():
                for tl in range(4):
                    t = 4 * c + tl
                    gt = state[tl // 2]
                    gc = (tl % 2) * 130
                    rs = sal_pool.tile([P, 2], f32, tag="rs")
                    rsrc = gt[:, gc:gc + 130].rearrange(
                        "p (hi d) -> p hi d", hi=2)[:, :, 64:65]
                    nc.vector.reciprocal(rs[:, :], rsrc)
                    for hi in range(2):
                        nc.vector.tensor_scalar_mul(
                            O[t][:, pr * P + hi * 64:pr * P + hi * 64 + 64],
                            gt[:, gc + hi * 65:gc + hi * 65 + 64],
                            rs[:, hi:hi + 1])

            steps = [alloc]
            steps += [(lambda j=j: do_j(j)) for j in range(8 * c + 8)]
            steps.append(norm)
            return steps

        # ---- Phase B: Q projection per pair + attention chunk 0 ----
        with tc.tile_pool(name="wqkv", bufs=2) as wqkv_pool, \
             tc.tile_pool(name="kqps", bufs=2, space="PSUM") as kqps:
            for pr in range(NPAIR):
                wq_sb = wqkv_pool.tile([P, 1024], bf16, tag="wqk")
                nc.sync.dma_start(out=wq_sb[:, :],
                                  in_=wq.ap()[:, pr * 1024:(pr + 1) * 1024])
                for qh in range(2):
                    pq = kqps.tile([P, 512], f32, tag="kq")
                    for kt in range(NKT):
                        qrhs = HT[kt][:, :].rearrange(
                            "p (t par d) -> p t par d", t=8, par=2)[
                                :, qh * 4:(qh + 1) * 4, 1:2, :]
                        nc.tensor.matmul(
                            pq[:, :], wq_sb[:, kt * P:(kt + 1) * P],
                            qrhs, start=(kt == 0), stop=(kt == NKT - 1))
                    nc.vector.tensor_scalar(
                        QT[pr][:, qh * 512:(qh + 1) * 512], pq[:, :],
                        qb_sb[:, pr:pr + 1], 0.125, ALU.add, ALU.mult)
                for s in attn_pr_steps(0, pr):
                    s()
        ht_es.close()

        # ---- shared FFN/Wo/transpose psum pool + weight streams ----
        fps = attn_es.enter_context(tc.tile_pool(name="fps", bufs=2, space="PSUM"))
        ot_pool = attn_es.enter_context(tc.tile_pool(name="otp", bufs=2))
        ln2p = attn_es.enter_context(tc.tile_pool(name="ln2", bufs=2))
        z2pool = attn_es.enter_context(tc.tile_pool(name="z2", bufs=2))
        h2t_pool = attn_es.enter_context(tc.tile_pool(name="h2t", bufs=1))
        ut_pool = attn_es.enter_context(tc.tile_pool(name="ut", bufs=ND))
        w1_pool = attn_es.enter_context(tc.tile_pool(name="w1s", bufs=3))
        w2_pool = attn_es.enter_context(tc.tile_pool(name="w2s", bufs=4))
        H2T = [h2t_pool.tile([P, 1024], bf16, tag=f"h2t{i}", name=f"h2t{i}")
               for i in range(NKT)]
        UT = [ut_pool.tile([P, 512], bf16, tag="ut", name=f"ut{i}") for i in range(ND)]

        def wo_t(t):
            """O[t] -> OT -> Wo -> X[t] residual (sps-pool psum)."""
            ot = ot_pool.tile([P, 1024], bf16, tag="ot")
            for kt in range(NKT):
                tp = sps.tile([P, 512], bf16, tag="sps", name="tpo")
                nc.tensor.transpose(tp[:, 0:P], O[t][:, kt * P:(kt + 1) * P],
                                    ident_sb[:, :])
                nc.scalar.copy(out=ot[:, kt * P:(kt + 1) * P],
                               in_=tp[:, 0:P])
            for hf in range(2):
                pw = sps.tile([P, 512], f32, tag="sps", name="pw")
                for kt in range(NKT):
                    nc.tensor.matmul(
                        pw[:, :], ot[:, kt * P:(kt + 1) * P],
                        wo_sb[:, kt * 1024 + hf * 512:kt * 1024 + (hf + 1) * 512],
                        start=(kt == 0), stop=(kt == NKT - 1))
                xsl = X[t][:, hf * 512:(hf + 1) * 512]
                nc.vector.tensor_add(xsl, pw[:, :], xsl)
                if flags["bo"]:
                    nc.vector.tensor_add(xsl, xsl, bo_sb[:, hf * 512:(hf + 1) * 512])

        def ln2_t(t):
            """X[t] -> LN2 -> z2 -> H2T columns (sps-pool psum)."""
            z2 = ln_tile(X[t], ln2p, z2pool)
            c, tl = t // 4, t % 4
            for kt in range(NKT):
                tp = sps.tile([P, 512], bf16, tag="sps", name="tpz")
                nc.tensor.transpose(tp[:, 0:P], z2[:, kt * P:(kt + 1) * P],
                                    ident_sb[:, :])
                nc.scalar.copy(
                    out=H2T[kt][:, c * 512 + tl * P:c * 512 + (tl + 1) * P],
                    in_=tp[:, 0:P])

        def ffn_steps(tch, chs=(0, 1), do_w1=True, dual_q=False):
            """Closures for FFN on token chunk tch (512 tokens)."""
            steps = []
            state = {}

            def w1_d(d):
                w1_sb = w1_pool.tile([P, 1024], bf16, tag="w1")
                nc.sync.dma_start(out=w1_sb[:, :],
                                  in_=w1.ap()[:, d * 1024:(d + 1) * 1024])
                pu = fps.tile([P, 512], f32, tag="fps")
                for kt in range(NKT):
                    nc.tensor.matmul(
                        pu[:, :], w1_sb[:, kt * P:(kt + 1) * P],
                        H2T[kt][:, tch * 512:(tch + 1) * 512],
                        start=(kt == 0), stop=(kt == NKT - 1))
                nc.vector.tensor_scalar(UT[d][:, :], pu[:, :],
                                        b1_sb[:, d:d + 1], 0.0,
                                        ALU.add, ALU.max)

            def w2_start():
                state["y0"] = fps.tile([P, 512], f32, tag="fps", name="y0")
                state["y1"] = fps.tile([P, 512], f32, tag="fps", name="y1")

            def w2_dp(ch, tp_, dp):
                w2_sb = w2_pool.tile([P, 2, 512], bf16, tag="w2")
                qeng = nc.scalar if (dual_q and dp % 2) else nc.sync
                qeng.dma_start(
                    out=w2_sb[:, :, :],
                    in_=w2.ap()[:, :].rearrange("p (d n) -> p d n", d=ND)[
                        :, 2 * dp:2 * dp + 2, ch * 512:(ch + 1) * 512])
                for dk in range(2):
                    d = 2 * dp + dk
                    for k in range(2):
                        tt = tp_ * 2 + k
                        nc.tensor.matmul(
                            state[f"y{k}"][:, :],
                            UT[d][:, tt * P:(tt + 1) * P], w2_sb[:, dk:dk + 1, :],
                            start=(d == 0), stop=(d == ND - 1))

            def w2_end(ch, tp_):
                for k in range(2):
                    t = tch * 4 + tp_ * 2 + k
                    xsl = X[t][:, ch * 512:(ch + 1) * 512]
                    nc.vector.tensor_add(xsl, state[f"y{k}"][:, :], xsl)
                    if flags["b2"]:
                        nc.vector.tensor_add(
                            xsl, xsl, b2_sb[:, ch * 512:(ch + 1) * 512])

            if do_w1:
                for d in range(ND):
                    steps.append(lambda d=d: w1_d(d))
            for ch in chs:
                for tp_ in range(2):
                    steps.append(w2_start)
                    for dp in range(ND // 2):
                        steps.append(lambda ch=ch, tp_=tp_, dp=dp: w2_dp(ch, tp_, dp))
                    steps.append(lambda ch=ch, tp_=tp_: w2_end(ch, tp_))
            return steps

        # ---- Phase C: Wo + LN2 for chunk 0, software-pipelined ----
        wo_t(0)
        wo_t(1)
        ln2_t(0)
        wo_t(2)
        ln2_t(1)
        wo_t(3)
        ln2_t(2)
        ln2_t(3)

        # ---- Phase D: FFN(tch0, W1+W2ch0) interleaved with attn chunk 1 ----
        f_steps = ffn_steps(0, chs=(0,))
        a_steps = []
        for pr in range(NPAIR):
            a_steps += attn_pr_steps(1, pr)
        fi = ai = 0
        while fi < len(f_steps) or ai < len(a_steps):
            if fi < len(f_steps):
                f_steps[fi]()
                fi += 1
            if ai < len(a_steps):
                a_steps[ai]()
                ai += 1
            if ai < len(a_steps):
                a_steps[ai]()
                ai += 1

        # ---- Phase E: Wo + LN2 chunk 1 (sps psum) || FFN0 W2 ch1 (fps) ----
        e_steps = [lambda: wo_t(4), lambda: wo_t(5), lambda: ln2_t(4),
                   lambda: wo_t(6), lambda: ln2_t(5), lambda: wo_t(7),
                   lambda: ln2_t(6), lambda: ln2_t(7)]
        f2_steps = ffn_steps(0, chs=(1,), do_w1=False, dual_q=True)
        fi = ei = 0
        while fi < len(f2_steps) or ei < len(e_steps):
            for _ in range(5):
                if fi < len(f2_steps):
                    f2_steps[fi]()
                    fi += 1
            if ei < len(e_steps):
                e_steps[ei]()
                ei += 1
        for t in range(4):
            nc.sync.dma_start(out=out.ap()[t * P:(t + 1) * P, :], in_=X[t][:, :])

        # ---- Phase F: FFN(tch1): W1 pipelined with W2ch0, then W2ch1 ----
        tch = 1

        def f_w1(d):
            w1_sb = w1_pool.tile([P, 1024], bf16, tag="w1")
            nc.sync.dma_start(out=w1_sb[:, :],
                              in_=w1.ap()[:, d * 1024:(d + 1) * 1024])
            pu = fps.tile([P, 512], f32, tag="fps")
            for kt in range(NKT):
                nc.tensor.matmul(
                    pu[:, :], w1_sb[:, kt * P:(kt + 1) * P],
                    H2T[kt][:, tch * 512:(tch + 1) * 512],
                    start=(kt == 0), stop=(kt == NKT - 1))
            nc.scalar.activation(UT[d][:, :], pu[:, :], FT.Relu,
                                 bias=b1_sb[:, d:d + 1], scale=1.0)

        def f_w2grp(ch, d, ya, yb, w2_sb, dk, dq=False):
            for k in range(2):
                for tp_ in range(2):
                    tt = tp_ * 2 + k
                    y = (ya, yb)[tp_]
                    nc.tensor.matmul(
                        y[:, k * 512:(k + 1) * 512],
                        UT[d][:, tt * P:(tt + 1) * P], w2_sb[:, dk:dk + 1, :],
                        start=(d == 0), stop=(d == ND - 1))

        def f_w2dma(ch, dp, dq=False):
            w2_sb = w2_pool.tile([P, 2, 512], bf16, tag="w2")
            qeng = nc.scalar if (dq and dp % 2) else nc.sync
            qeng.dma_start(
                out=w2_sb[:, :, :],
                in_=w2.ap()[:, :].rearrange("p (d n) -> p d n", d=ND)[
                    :, 2 * dp:2 * dp + 2, ch * 512:(ch + 1) * 512])
            return w2_sb

        def f_w2end(ch, ya, yb):
            for tp_ in range(2):
                for k in range(2):
                    t = tch * 4 + tp_ * 2 + k
                    xsl = X[t][:, ch * 512:(ch + 1) * 512]
                    nc.vector.tensor_add(xsl, (ya, yb)[tp_][:, k * 512:(k + 1) * 512], xsl)
                    if flags["b2"]:
                        nc.vector.tensor_add(
                            xsl, xsl, b2_sb[:, ch * 512:(ch + 1) * 512])
                    if ch == 1:
                        nc.sync.dma_start(
                            out=out.ap()[t * P:(t + 1) * P, :], in_=X[t][:, :])

        ya = sps.tile([P, 1024], f32, tag="sps", name="ya")
        yb = sps.tile([P, 1024], f32, tag="sps", name="yb")
        f_w1(0)
        f_w1(1)
        w2t = {}
        for d in range(ND):
            if d % 2 == 0:
                w2t[d // 2] = f_w2dma(0, d // 2)
            f_w2grp(0, d, ya, yb, w2t[d // 2], d % 2)
            if d + 2 < ND:
                f_w1(d + 2)
        f_w2end(0, ya, yb)
        ya2 = sps.tile([P, 1024], f32, tag="sps", name="ya2")
        yb2 = sps.tile([P, 1024], f32, tag="sps", name="yb2")
        for dp in range(ND // 2):
            w2_sb = f_w2dma(1, dp, dq=True)
            for dk in range(2):
                f_w2grp(1, 2 * dp + dk, ya2, yb2, w2_sb, dk)
        f_w2end(1, ya2, yb2)
        attn_es.close()

    nc.compile()
    return nc


_CACHE = {}


def _prep(inputs):
    """Host-side preprocessing: fold LN affine into weights, tile/cast, shard."""
    x = np.asarray(inputs["x"], np.float32)
    Wq = np.asarray(inputs["Wq"], np.float32)
    Wk = np.asarray(inputs["Wk"], np.float32)
    Wv = np.asarray(inputs["Wv"], np.float32)
    Wo = np.asarray(inputs["Wo"], np.float32)
    bo = np.asarray(inputs["bo"], np.float32)
    W1 = np.asarray(inputs["W1"], np.float32)
    b1 = np.asarray(inputs["b1"], np.float32)
    W2 = np.asarray(inputs["W2"], np.float32)
    b2 = np.asarray(inputs["b2"], np.float32)
    g1 = np.asarray(inputs["g1"], np.float32)
    be1 = np.asarray(inputs["be1"], np.float32)
    g2 = np.asarray(inputs["g2"], np.float32)
    be2 = np.asarray(inputs["be2"], np.float32)

    Wq_g = (Wq * g1[None, :, None]).astype(BF16)   # [16,1024,64]
    Wk_g = (Wk * g1[None, :, None]).astype(BF16)
    Wv_g = (Wv * g1[None, :, None]).astype(BF16)
    qb = np.einsum('c,hcd->hd', be1, Wq_g.astype(np.float32))  # [16,64]
    kb = np.einsum('c,hcd->hd', be1, Wk_g.astype(np.float32))
    vb = np.einsum('c,hcd->hd', be1, Wv_g.astype(np.float32))
    if np.abs(vb).max() > 0:
        raise NotImplementedError("nonzero folded V bias not supported")

    def lhsT_pack(wflat):  # [1024 c, 1024 m] -> [128, (pair, kt, 128)]
        return np.ascontiguousarray(
            wflat.reshape(8, 128, 8, 128).transpose(1, 2, 0, 3).reshape(128, 8192))

    def rhs_pack(wflat):   # [1024 k, 1024 n] -> [128, (kt, 1024)]
        return np.ascontiguousarray(
            wflat.reshape(8, 128, 1024).transpose(1, 0, 2).reshape(128, 8192))

    wq_h = lhsT_pack(Wq_g.transpose(1, 0, 2).reshape(1024, 1024))
    wk_h = lhsT_pack(Wk_g.transpose(1, 0, 2).reshape(1024, 1024))
    wv_h = rhs_pack(Wv_g.transpose(1, 0, 2).reshape(1024, 1024))
    wo_h = rhs_pack(Wo.astype(BF16))
    W1_g = (W1 * g2[:, None]).astype(BF16)         # [1024, 4096]
    b1p = b1 + be2 @ W1_g.astype(np.float32)
    w1_h = np.ascontiguousarray(
        W1_g.reshape(8, 128, 32, 128).transpose(1, 2, 0, 3).reshape(128, 32768))
    w2_h = np.ascontiguousarray(
        W2.astype(BF16).reshape(32, 128, 1024).transpose(1, 0, 2).reshape(128, 32768))

    qb_t = np.zeros((128, 8), np.float32)
    kb_t = np.zeros((128, 8), np.float32)
    for pr in range(8):
        qb_t[0:64, pr] = qb[2 * pr]
        qb_t[64:128, pr] = qb[2 * pr + 1]
        kb_t[0:64, pr] = kb[2 * pr]
        kb_t[64:128, pr] = kb[2 * pr + 1]
    b1_t = np.ascontiguousarray(b1p.reshape(32, 128).T.astype(np.float32))
    bo_t = np.broadcast_to(bo, (128, 1024)).astype(np.float32).copy()
    b2_t = np.broadcast_to(b2, (128, 1024)).astype(np.float32).copy()

    triu = np.triu(np.ones((128, 128), np.float32))
    ident = np.eye(128, dtype=np.float32).astype(BF16)

    flags = {"bo": bool(np.abs(bo).max() > 0), "b2": bool(np.abs(b2).max() > 0)}

    shared = dict(wq=wq_h, wk=wk_h, wv=wv_h, wo=wo_h, w1=w1_h, w2=w2_h,
                  identd=ident, qbias=qb_t, kbias=kb_t,
                  b1p=b1_t, bo_row=bo_t, b2_row=b2_t)
    in_maps = []
    for core in range(8):
        b, par = core // 2, core % 2
        xb = x[b].reshape(16, 128, 1024)
        if par == 0:
            # swap even/odd blocks so own (even-global) blocks sit at odd slots
            perm = [i + 1 if i % 2 == 0 else i - 1 for i in range(16)]
            xw = np.ascontiguousarray(xb[perm].reshape(2048, 1024))
            m1 = np.zeros((128, 128), np.float32)
        else:
            xw = np.ascontiguousarray(xb.reshape(2048, 1024))
            m1 = np.ones((128, 128), np.float32)
        mk = np.concatenate([m1, triu], axis=1).astype(BF16)
        in_maps.append({"xkv": xw, "masks": mk, **shared})
    return in_maps, flags


def _get_nc(flags):
    key = tuple(sorted(flags.items()))
    if key not in _CACHE:
        _CACHE[key] = _build(flags)
    return _CACHE[key]


def run(inputs, **kw):
    in_maps, flags = _prep(inputs)
    nc = _get_nc(flags)
    res = run_bass_kernel_spmd(nc, in_maps, core_ids=list(range(8)), **kw)
    x = np.asarray(inputs["x"], np.float32)
    outf = np.zeros_like(x)
    for core in range(8):
        b, par = core // 2, core % 2
        r = np.asarray(res.results[core]["out"], np.float32)
        for t in range(8):
            g = 2 * t + par
            outf[b, g * 128:(g + 1) * 128] = r[t * 128:(t + 1) * 128]
    return outf, res


def kernel(**inputs):
    outf, _ = run(inputs)
    return outf
